# revision 51
# baseline (speedup 1.0000x reference)
"""Causal self-attention TRN2 kernel, tensor-parallel over heads on 8 NeuronCores.

Model (N=4096 tokens, D=2048, H=16 heads, HD=128):
    q = x @ Wq.T + bq ; k = x @ Wk.T + bk ; v = x @ Wv.T + bv   (per head)
    attn = softmax(q k^T / sqrt(HD) + causal_mask)
    y = concat_h(attn @ v) @ Wo.T + bo
Sharding: core c owns heads {2c, 2c+1} -> computes its QKV column slices,
attention for its heads, and a partial output projection
y_c = out_heads_c @ Wo[:, cols_c].T (+ bias/8).  Host sums the 8 partials.

Per-core kernel layout choices:
  * x is fed transposed (xT: D x N, fp16); an fp8 copy is cast on-device
    (DVE) for the Q/K projections, which run entirely in fp8e4 DoubleRow
    (256-deep contraction, 2x PE rate).  V stays fp16: fp8 V-projection
    noise passes straight through peaked attention rows (measured 3e-2).
  * q,k are produced directly transposed per head: qT/kT = (HD x N), fp16.
  * scores are computed transposed: sT[k,q] = kT_blk.T @ qT_blk, so the
    PV matmul needs no transposes at all: oT += v_blk.T @ exp(sT).
  * softmax skips the max-subtraction (scores are O(1); exp cannot
    overflow) -> row sums come from a ones-vector matmul on the PE, and
    1/rowsum is applied to oT (broadcast along partitions).
  * causality: key blocks entirely above the diagonal are skipped; blocks
    straddling the diagonal get -1e9 added via a precomputed triangular
    strip before the exp.
  * the attention pipeline works in key-block PAIRS: the two score matmuls
    of a pair share one 2-bank PSUM tile, so exp runs as a single 1024-wide
    ACT instruction (the ACT engine is the phase-2 co-bottleneck).
  * for query blocks >= QB8_START both exp(scores) and v are kept in fp8,
    so PV and the row-sum run as DoubleRow pair passes (2 key blocks per
    512-cycle pass).  Early rows (peaked attention) stay fp16.
  * v bias folds into the output bias exactly (attn rows sum to 1):
    y += (bo + Wo @ bv) / ncores  added on-device per core.
  * y partials leave the device as fp16 (host accumulates in fp32).
"""

from collections import deque
from contextlib import ExitStack

import numpy as np
import ml_dtypes

import concourse.bass as bass
import concourse.tile as tile
from concourse import bacc
from concourse import mybir
from concourse.bass_utils import run_bass_kernel_spmd
from concourse.masks import make_identity

N, D, H, HD = 4096, 2048, 16, 128
NCORES = 8
HPC = H // NCORES            # heads per core (2)
CD = HPC * HD                # per-core head-dim slice (256)
SCALE = 1.0 / float(np.sqrt(HD))
NEG = -1e9
W8SCALE = 16.0               # power-of-2 prescale keeping fp8 weights normal

QB = 512                     # query block (free dim of moving operands)
KB = 128                     # key block (partition dim of scores)
NQB = N // QB                # 8
KC8 = D // 256               # 256-deep contraction chunks (8)

F32 = mybir.dt.float32
F16 = mybir.dt.float16
F8 = mybir.dt.float8e4
DR = mybir.MatmulPerfMode.DoubleRow

# Query blocks >= QB8_START keep exp(scores) and v in fp8e4 so PV and the
# row-sum matmuls run in DoubleRow mode (2 key blocks per pass).  Early rows
# have peaked attention where quantization hurts; late rows average over many
# keys, so e4m3 noise washes out.
QB8_START = 1
# Query blocks >= OPROJ8_START run the output projection as a single fp8e4
# DoubleRow pass (256-deep: both heads at once).  The max error lives in the
# early rows (QK fp8 noise through peaked attention); fp8 outproj noise on
# later rows stays below it (verified against the reference inputs).
OPROJ8_START = 2
# V projection runs fp8e4 DoubleRow for token stripes >= 2: those v rows are
# only ever consumed through the (already fp8) v8 path by diffuse late query
# rows.  Stripes 0-1 keep the fp16 projection for the peaked early rows.
V8_STRIPE_START = 2


def build_nc(causal: bool = True) -> bass.Bass:
    nc = bacc.Bacc(None)

    # fp16 x is only needed for the fp16 V projection of the first
    # V8_STRIPE_START token stripes; everything else consumes the fp8 copy
    xT = nc.declare_dram_parameter(
        "xT", [D, V8_STRIPE_START * QB], F16, isOutput=False)
    xT8 = nc.declare_dram_parameter("xT8", [D, N], F8, isOutput=False)
    wqT8 = nc.declare_dram_parameter("wqT8", [D, CD], F8, isOutput=False)
    wkT8 = nc.declare_dram_parameter("wkT8", [D, CD], F8, isOutput=False)
    wvT = nc.declare_dram_parameter("wvT", [D, CD], F16, isOutput=False)
    wvT8 = nc.declare_dram_parameter("wvT8", [D, CD], F8, isOutput=False)
    woT = nc.declare_dram_parameter("woT", [CD, D], F16, isOutput=False)
    woT8 = nc.declare_dram_parameter("woT8", [CD, D], F8, isOutput=False)
    bq = nc.declare_dram_parameter("bq", [CD], F32, isOutput=False)
    bk = nc.declare_dram_parameter("bk", [CD], F32, isOutput=False)
    maskT = None
    if not causal:
        maskT = nc.declare_dram_parameter("maskT", [N, N], F32, isOutput=False)
    y = nc.declare_dram_parameter("y", [N, D], F16, isOutput=True)

    # fp16 v is only consumed by the fp16 PV path (query blocks < QB8_START),
    # which under causality only ever reads keys < QB8_START * QB.
    NV16 = QB8_START * QB if causal else N

    with tile.TileContext(nc) as tc, tc.tile_pool(name="persist", bufs=1) as persist:
        # ---------------- setup: weights, biases, constants -------------
        # Wo^T slice: (CD, D) -> per head (128, D); DMA'd later (scalar
        # queue, after the projection weights -- it is first needed at the
        # output projection, deep into phase 2).  fp16 for early query
        # blocks, fp8 (DoubleRow, both heads per pass) for the rest.
        wo_sb = persist.tile([128, HPC * D], F16, tag="wo")
        wo8_sb = persist.tile([128, HPC * D], F8, tag="wo8")
        # q/k biases: (CD,) -> (128, HPC), partition = dim within head
        bq_sb = persist.tile([128, HPC], F32, tag="bq")
        bk_sb = persist.tile([128, HPC], F32, tag="bk")
        nc.sync.dma_start(out=bq_sb[:], in_=bq[:].rearrange("(h p) -> p h", p=128))
        nc.sync.dma_start(out=bk_sb[:], in_=bk[:].rearrange("(h p) -> p h", p=128))
        # identity for PE transposes
        ident = persist.tile([128, 128], F16, tag="ident")
        # full ones matrix: row-sum matmul with this stationary operand
        # broadcasts the sum to all 128 output partitions at no extra cost
        ones = persist.tile([128, 128], F16, tag="ones")
        nc.vector.memset(ones[:], 1.0)
        # fp8 ones pair for DoubleRow row-sum matmuls (256-deep contraction)
        ones8 = persist.tile([128, 256], F8, tag="ones8")
        nc.vector.memset(ones8[:], 1.0)
        def emit_setup_selects():
            make_identity(nc, ident[:])

        # PE warm-up: dependency-free matmuls fill the DMA-startup window
        # and push the HAM clock gate to full rate before real work
        warm_sb = persist.tile([128, QB], F16, tag="warm")
        nc.vector.memset(warm_sb[:], 0.0)

        # Persistent activations: qT/kT per head (HD x N) fp16; v per head
        # stored (128, nblk*128 + hd) i.e. key-block-major with hd minor, in
        # fp16 (first NV16 keys only) and fp8 (all keys, feeds DR PV pairs).
        qT = [persist.tile([128, N], F16, tag=f"qT{h}", name=f"qT{h}")
              for h in range(HPC)]
        kT = [persist.tile([128, N], F16, tag=f"kT{h}", name=f"kT{h}")
              for h in range(HPC)]
        v_sb = [persist.tile([128, NV16], F16, tag=f"v{h}", name=f"v{h}")
                for h in range(HPC)]
        v8_sb = [persist.tile([128, N], F8, tag=f"v8{h}", name=f"v8{h}")
                 for h in range(HPC)]

        # ---------------- phase 1: QKV projections ----------------------
        # Q/K run in fp8e4 DoubleRow mode (256-deep contraction per pass,
        # 2x PE rate) on a device-cast fp8 copy of x; V stays fp16.
        with ExitStack() as p1:
            wproj = p1.enter_context(tc.tile_pool(name="wproj", bufs=1))
            xin = p1.enter_context(tc.tile_pool(name="xin", bufs=8))
            x8in = p1.enter_context(tc.tile_pool(name="x8in", bufs=8))
            vtpool = p1.enter_context(tc.tile_pool(name="vt", bufs=2))
            qkv_ps = p1.enter_context(tc.tile_pool(name="qkv_ps", bufs=6, space="PSUM"))
            tp_ps = p1.enter_context(tc.tile_pool(name="tp_ps", bufs=2, space="PSUM"))

            # Per-chunk weight tiles keep reader deps fine-grained: the first
            # matmul of chunk c only waits on chunk c's DMA, not the whole
            # weight array.  fp8 Q/K chunks [p, (j d)], contraction row
            # j*128+p; fp16 V chunks: two 128-deep subchunks [p, (g d)].
            wq8_c = [wproj.tile([128, 2 * CD], F8, tag=f"wq8_{c}", name=f"wq8_{c}")
                     for c in range(KC8)]
            wk8_c = [wproj.tile([128, 2 * CD], F8, tag=f"wk8_{c}", name=f"wk8_{c}")
                     for c in range(KC8)]
            wv8_c = [wproj.tile([128, 2 * CD], F8, tag=f"wv8_{c}", name=f"wv8_{c}")
                     for c in range(KC8)]
            wv_c = [wproj.tile([128, 2 * CD], F16, tag=f"wv_{c}", name=f"wv_{c}")
                    for c in range(KC8)]
            # weight DMAs go on the scalar queue so the sync queue is free to
            # carry half of the x stream from the very start
            for c in range(KC8):
                for w_sb, w_dram in ((wq8_c[c], wqT8), (wk8_c[c], wkT8),
                                     (wv8_c[c], wvT8)):
                    nc.scalar.dma_start(
                        out=w_sb[:].rearrange("p (j d) -> p j d", j=2),
                        in_=w_dram[c * 256:(c + 1) * 256, :].rearrange(
                            "(j p) d -> p j d", p=128),
                    )
                nc.scalar.dma_start(
                    out=wv_c[c][:].rearrange("p (g d) -> p g d", g=2),
                    in_=wvT[c * 256:(c + 1) * 256, :].rearrange(
                        "(g p) d -> p g d", p=128),
                )
            warm_ps = tp_ps.tile([128, QB], F32, tag="tp", name="warm_ps")
            for wi in range(20):
                nc.tensor.matmul(
                    warm_ps[:], lhsT=ones[:], rhs=warm_sb[:],
                    start=(wi == 0), stop=(wi == 19),
                )

            for nb in range(N // QB):  # 512-token stripes
                # psum tiles for qT/kT/vT of both heads
                pss = {}
                for nm in ("q", "k", "v"):
                    for h in range(HPC):
                        pss[nm, h] = qkv_ps.tile(
                            [128, QB], F32, tag="qkv", name=f"ps_{nm}{h}_{nb}"
                        )
                wnames = (((wq8_c, "q"), (wk8_c, "k"), (wv8_c, "v"))
                          if nb >= V8_STRIPE_START
                          else ((wq8_c, "q"), (wk8_c, "k")))
                for c in range(KC8):  # 256-deep contraction chunks
                    # alternate x chunks across two DMA queues for bandwidth
                    dma_eng = nc.gpsimd if c % 2 == 0 else nc.sync
                    x8t = x8in.tile([128, 2 * QB], F8, tag="x8",
                                    name=f"x8_{nb}_{c}")
                    dma_eng.dma_start(
                        out=x8t[:].rearrange("p (j q) -> p j q", j=2),
                        in_=xT8[c * 256:(c + 1) * 256,
                                nb * QB:(nb + 1) * QB].rearrange(
                            "(j p) q -> p j q", p=128),
                    )
                    x8_ap = x8t[:].rearrange("p (j q) -> p j q", j=2)
                    for w_c, nm in wnames:
                        w_ap = w_c[c][:].rearrange("p (j d) -> p j d", j=2)
                        for h in range(HPC):
                            nc.tensor.matmul(
                                pss[nm, h][:],
                                lhsT=w_ap[:, :, h * HD:(h + 1) * HD],
                                rhs=x8_ap,
                                start=(c == 0),
                                stop=(c == KC8 - 1),
                                perf_mode=DR,
                            )
                if nb < V8_STRIPE_START:
                    # fp16 V runs as a second sweep so the fp8 Q/K stream
                    # (whose x8 tiles land first) is never blocked on the
                    # bigger fp16 x transfers during the DMA ramp
                    for c in range(KC8):
                        xt = xin.tile([128, 2 * QB], F16, tag="xt",
                                      name=f"xt_{nb}_{c}")
                        dma_eng = nc.gpsimd if c % 2 == 0 else nc.sync
                        dma_eng.dma_start(
                            out=xt[:].rearrange("p (g q) -> p g q", g=2),
                            in_=xT[c * 256:(c + 1) * 256,
                                   nb * QB:(nb + 1) * QB].rearrange(
                                "(g p) q -> p g q", p=128),
                        )
                        for g in range(2):
                            for h in range(HPC):
                                nc.tensor.matmul(
                                    pss["v", h][:],
                                    lhsT=wv_c[c][:, g * CD + h * HD:
                                                 g * CD + (h + 1) * HD],
                                    rhs=xt[:, g * QB:(g + 1) * QB],
                                    start=(c == 0 and g == 0),
                                    stop=(c == KC8 - 1 and g == 1),
                                )
                if nb == 0:
                    # gpsimd setup ops, emitted after the first stripe's x DMAs
                    # so they don't block the queue head (ident is first needed
                    # by the v transposes just below)
                    emit_setup_selects()
                for nm, b_sb, dst in (("q", bq_sb, qT), ("k", bk_sb, kT)):
                    for h in range(HPC):
                        # 1/W8SCALE undoes the fp8 weight prescale
                        nc.scalar.activation(
                            out=dst[h][:, nb * QB:(nb + 1) * QB],
                            in_=pss[nm, h][:],
                            func=mybir.ActivationFunctionType.Identity,
                            bias=b_sb[:, h:h + 1],
                            scale=1.0 / W8SCALE,
                        )
                # v: evacuate vT (undoing the fp8 prescale for DR stripes),
                # then PE-transpose into (n, hd) layout, stored fp8 (all
                # keys) + fp16 (early keys)
                for h in range(HPC):
                    vt = vtpool.tile([128, QB], F16, tag="vt", name=f"vt_{nb}_{h}")
                    if nb >= V8_STRIPE_START:
                        nc.vector.tensor_scalar(
                            out=vt[:], in0=pss["v", h][:],
                            scalar1=1.0 / W8SCALE, scalar2=None,
                            op0=mybir.AluOpType.mult,
                        )
                    else:
                        nc.vector.tensor_copy(out=vt[:], in_=pss["v", h][:])
                    for s in range(QB // 128):
                        tp = tp_ps.tile([128, 128], F16, tag="tp",
                                        name=f"tp_{nb}_{h}_{s}")
                        nc.tensor.transpose(
                            tp[:], vt[:, s * 128:(s + 1) * 128], ident[:]
                        )
                        nblk = nb * (QB // 128) + s
                        nc.vector.tensor_copy(
                            out=v8_sb[h][:, nblk * 128:(nblk + 1) * 128],
                            in_=tp[:],
                        )
                        if nblk * 128 < NV16:
                            nc.vector.tensor_copy(
                                out=v_sb[h][:, nblk * 128:(nblk + 1) * 128],
                                in_=tp[:],
                            )

        # wo weight DMAs submit at the start of phase 2: they are first
        # needed ~30us later and would otherwise steal HBM bandwidth from
        # the x stream during the phase-1 ramp
        nc.scalar.dma_start(
            out=wo_sb[:].rearrange("p (h d) -> p h d", h=HPC),
            in_=woT[:].rearrange("(h p) d -> p h d", p=128),
        )
        nc.scalar.dma_start(
            out=wo8_sb[:].rearrange("p (h d) -> p h d", h=HPC),
            in_=woT8[:].rearrange("(h p) d -> p h d", p=128),
        )

        # ---------------- phase 2: attention + output projection --------
        with ExitStack() as p2:
            ptpool = p2.enter_context(tc.tile_pool(name="pt", bufs=8))
            otpool = p2.enter_context(tc.tile_pool(name="ot", bufs=6))
            ypool = p2.enter_context(tc.tile_pool(name="yout", bufs=12))
            small = p2.enter_context(tc.tile_pool(name="small", bufs=4))
            mtpool = p2.enter_context(tc.tile_pool(name="mt", bufs=4))
            # score-pair tiles: [128, 1024] fp32 = 2 PSUM banks each, so one
            # 1024-wide ACT exp covers both key blocks of a pair
            s_ps = p2.enter_context(tc.tile_pool(name="s_ps", bufs=2, space="PSUM"))
            o_ps = p2.enter_context(tc.tile_pool(name="o_ps", bufs=1, space="PSUM"))
            r_ps = p2.enter_context(tc.tile_pool(name="r_ps", bufs=1, space="PSUM"))
            y_ps = p2.enter_context(tc.tile_pool(name="y_ps", bufs=2, space="PSUM"))

            # Flat software-pipelined attention over key-block PAIRS:
            # scores/exp run SKEW pairs ahead of PV/rowsum, so the PE never
            # waits on the ACT exp latency (or the GpSimd diagonal-select
            # latency) -- including across head and query-block boundaries.
            # Output-projection pieces drip in between pairs to spread load.
            SKEW = 4
            units = []   # (qb, h, jp, npairs)
            for qb in range(NQB):
                npairs = (qb + 1) * (QB // KB) // 2 if causal else N // KB // 2
                for h in range(HPC):
                    for jp in range(npairs):
                        units.append((qb, h, jp, npairs))
            ready = []      # qblocks whose heads are normalized
            DELAY = 4       # pipeline pairs between normalize and outproj
            pending = deque()

            pts = {}
            o_psum = {}
            r_psum = {}
            oT_sb = {}

            def qoff_of(qb, nkb):
                # causal: columns q < off are fully masked for this key block;
                # skip them (exact -- their exp is 0)
                off = nkb * KB - qb * QB
                return max(0, off) if causal else 0

            def emit_front(qb, h, jp, npairs):
                fp8blk = causal and qb >= QB8_START
                sps = s_ps.tile([128, 2 * QB], F32, tag="s",
                                name=f"s_{qb}_{h}_{jp}")
                qoffs = []
                for j in range(2):
                    nkb = 2 * jp + j
                    qoff = qoff_of(qb, nkb)
                    qoffs.append(qoff)
                    w = QB - qoff
                    half = sps[:, j * QB:(j + 1) * QB]
                    nc.tensor.matmul(
                        half[:, qoff:],
                        lhsT=kT[h][:, nkb * KB:(nkb + 1) * KB],
                        rhs=qT[h][:, qb * QB + qoff:(qb + 1) * QB],
                        start=True,
                        stop=True,
                    )
                    if not causal:
                        mt = mtpool.tile([128, QB], F32, tag="mt",
                                         name=f"mt_{qb}_{h}_{jp}_{j}")
                        nc.sync.dma_start(
                            out=mt[:],
                            in_=maskT[nkb * KB:(nkb + 1) * KB,
                                      qb * QB:(qb + 1) * QB],
                        )
                        nc.vector.scalar_tensor_tensor(
                            out=half[:],
                            in0=mt[:],
                            scalar=1.0 / SCALE,
                            in1=half[:],
                            op0=mybir.AluOpType.mult,
                            op1=mybir.AluOpType.add,
                        )
                qoff0, qoff1 = qoffs
                if fp8blk:
                    pt2 = ptpool.tile([128, 2 * QB], F8, tag="pt8",
                                      name=f"pt8_{qb}_{h}_{jp}")
                    # one fused 1024-wide exp covers the whole pair; causal
                    # masking of diagonal blocks happens AFTER the exp as a
                    # triangular zero-fill on the fp8 tile (GpSimd, which is
                    # idle in phase 2 -- keeps ACT and DVE off this path)
                    nc.scalar.activation(
                        out=pt2[:], in_=sps[:],
                        func=mybir.ActivationFunctionType.Exp,
                        scale=SCALE,
                    )
                    for j in range(2):
                        off = (2 * jp + j) * KB - qb * QB
                        if causal and off >= 0:
                            nc.gpsimd.affine_select(
                                out=pt2[:, j * QB + qoff0:(j + 1) * QB],
                                in_=pt2[:, j * QB + qoff0:(j + 1) * QB],
                                compare_op=mybir.AluOpType.is_ge,
                                fill=0.0,
                                base=qoff0 - off,
                                pattern=[[1, QB - qoff0]],
                                channel_multiplier=-1,
                            )
                    return (pt2, qoff0)
                # fp16 path: two per-block exps into separate fp16 tiles
                res = []
                for j in range(2):
                    nkb = 2 * jp + j
                    qoff = qoffs[j]
                    w = QB - qoff
                    pt = ptpool.tile([128, QB], F16, tag="pt",
                                     name=f"pt_{qb}_{h}_{jp}_{j}")
                    nc.scalar.activation(
                        out=pt[:, :w], in_=sps[:, j * QB + qoff:(j + 1) * QB],
                        func=mybir.ActivationFunctionType.Exp,
                        scale=SCALE,
                    )
                    if causal and nkb * KB - qb * QB >= 0:
                        nc.gpsimd.affine_select(
                            out=pt[:, :w],
                            in_=pt[:, :w],
                            compare_op=mybir.AluOpType.is_ge,
                            fill=0.0,
                            base=0,
                            pattern=[[1, w]],
                            channel_multiplier=-1,
                        )
                    res.append(pt)
                return (res, None)

            def emit_outproj_piece(qb, qs, dc, pidx, tail=False):
                yps = y_ps.tile([128, QB], F32, tag="y",
                                name=f"y_{qb}_{qs}_{dc}")
                fp8piece = causal and qb >= OPROJ8_START
                if fp8piece:
                    # single DoubleRow pass: contraction over both heads'
                    # 128 oT dims at once (fp8 oT x fp8 Wo)
                    nc.tensor.matmul(
                        yps[:],
                        lhsT=oT_sb[qb][:].rearrange(
                            "p (j q) -> p j q", j=2)[:, :, qs * 128:(qs + 1) * 128],
                        rhs=wo8_sb[:].rearrange(
                            "p (j d) -> p j d", j=2)[:, :, dc * QB:(dc + 1) * QB],
                        start=True,
                        stop=True,
                        perf_mode=DR,
                    )
                else:
                    for h in range(HPC):
                        nc.tensor.matmul(
                            yps[:],
                            lhsT=oT_sb[qb, h][:, qs * 128:(qs + 1) * 128],
                            rhs=wo_sb[:, h * D + dc * QB: h * D + (dc + 1) * QB],
                            start=(h == 0),
                            stop=(h == HPC - 1),
                        )
                ysb = ypool.tile([128, QB], F16, tag="ysb",
                                 name=f"ys_{qb}_{qs}_{dc}")
                # psum evacuation on DVE (output bias is added on the host;
                # fp8 pieces also undo the Wo prescale); during the final
                # flush the exp stream is done, so ACT shares the evacuation
                # load and the y DMAs fan out over two queues
                scale = 1.0 / W8SCALE if fp8piece else 1.0
                if tail and pidx % 2 == 1:
                    nc.scalar.activation(
                        out=ysb[:], in_=yps[:],
                        func=mybir.ActivationFunctionType.Identity,
                        scale=scale,
                    )
                elif fp8piece:
                    nc.vector.tensor_scalar(
                        out=ysb[:], in0=yps[:], scalar1=scale,
                        scalar2=None, op0=mybir.AluOpType.mult,
                    )
                else:
                    nc.vector.tensor_copy(out=ysb[:], in_=yps[:])
                row0 = qb * QB + qs * 128
                dma_eng = nc.gpsimd if (tail and pidx % 2 == 1) else nc.sync
                dma_eng.dma_start(
                    out=y[row0:row0 + 128, dc * QB:(dc + 1) * QB], in_=ysb[:]
                )

            def emit_back(qb, h, jp, npairs):
                fp8blk = causal and qb >= QB8_START
                if jp == 0:
                    o_psum[qb, h] = o_ps.tile([128, QB], F32, tag="o",
                                              name=f"o_{qb}_{h}")
                    r_psum[qb, h] = r_ps.tile([128, QB], F32, tag="r",
                                              name=f"r_{qb}_{h}")
                pt, qoff0 = pts.pop((qb, h, jp))
                rbc = None
                if fp8blk:
                    # fp8 DoubleRow pair passes: PV and rowsum cover both key
                    # blocks in one 256-deep pass each.  The rowsum goes
                    # first, and on the last pair the reciprocal is emitted
                    # between rowsum and PV: the r bank frees one DVE-latency
                    # earlier, which is what the next head's first rowsum
                    # waits on.
                    pt2_ap = pt[:].rearrange("p (j q) -> p j q", j=2)
                    nc.tensor.matmul(
                        r_psum[qb, h][:, qoff0:],
                        lhsT=ones8[:].rearrange("p (j c) -> p j c", j=2),
                        rhs=pt2_ap[:, :, qoff0:],
                        start=(jp == 0),
                        stop=(jp == npairs - 1),
                        perf_mode=DR,
                    )
                    if jp == npairs - 1:
                        rbc = small.tile([128, QB], F32, tag="rbc",
                                         name=f"rb_{qb}_{h}")
                        nc.vector.reciprocal_approx_fast(
                            out=rbc[:], in_=r_psum.pop((qb, h))[:])
                    nc.tensor.matmul(
                        o_psum[qb, h][:, qoff0:],
                        lhsT=v8_sb[h][:, jp * 256:(jp + 1) * 256].rearrange(
                            "p (j d) -> p j d", j=2),
                        rhs=pt2_ap[:, :, qoff0:],
                        start=(jp == 0),
                        stop=(jp == npairs - 1),
                        perf_mode=DR,
                    )
                else:
                    for j in range(2):
                        nkb = 2 * jp + j
                        qoff = qoff_of(qb, nkb)
                        w = QB - qoff
                        nc.tensor.matmul(
                            r_psum[qb, h][:, qoff:],
                            lhsT=ones[:],
                            rhs=pt[j][:, :w],
                            start=(nkb == 0),
                            stop=(nkb == 2 * npairs - 1),
                        )
                        if nkb == 2 * npairs - 1:
                            rbc = small.tile([128, QB], F32, tag="rbc",
                                             name=f"rb_{qb}_{h}")
                            nc.vector.reciprocal_approx_fast(
                                out=rbc[:], in_=r_psum.pop((qb, h))[:])
                        nc.tensor.matmul(
                            o_psum[qb, h][:, qoff:],
                            lhsT=v_sb[h][:, nkb * KB:(nkb + 1) * KB],
                            rhs=pt[j][:, :w],
                            start=(nkb == 0),
                            stop=(nkb == 2 * npairs - 1),
                        )
                if jp == npairs - 1:
                    # ~18-bit approx reciprocal is plenty for normalization;
                    # one DVE multiply then normalizes oT
                    if causal and qb >= OPROJ8_START:
                        # both heads' normalized oT land in one fp8 tile so
                        # the output projection runs as a single DoubleRow
                        # pass (256-deep contraction: h0+h1)
                        if h == 0:
                            oT_sb[qb] = otpool.tile(
                                [128, 2 * QB], F8, tag="ot8", name=f"ot8_{qb}")
                        nc.vector.tensor_mul(
                            oT_sb[qb][:, h * QB:(h + 1) * QB],
                            o_psum.pop((qb, h))[:], rbc[:])
                    else:
                        ot = otpool.tile([128, QB], F16, tag="ot",
                                         name=f"ot_{qb}_{h}")
                        nc.vector.tensor_mul(
                            ot[:], o_psum.pop((qb, h))[:], rbc[:])
                        oT_sb[qb, h] = ot
                    if h == HPC - 1:
                        ready.append(qb)

            ready_at = {}
            pidx = 0
            for i, u in enumerate(units):
                pts[u[:3]] = emit_front(*u)
                if i >= SKEW:
                    n_ready = len(ready)
                    emit_back(*units[i - SKEW])
                    if len(ready) > n_ready:
                        ready_at[ready[-1]] = i
                while ready and i - ready_at[ready[0]] >= DELAY:
                    qb = ready.pop(0)
                    for qs in range(QB // 128):
                        for dc in range(D // QB):
                            pending.append((qb, qs, dc))
                # drip outproj pieces between pairs to spread the load --
                # but not right before a head-boundary back: the boundary's
                # reciprocal must not queue behind drip evacuations on DVE
                nxt = units[i - SKEW + 1] if SKEW <= i + 1 < len(units) + SKEW \
                    and i - SKEW + 1 < len(units) else None
                if not (nxt and nxt[2] == nxt[3] - 1):
                    for _ in range(2):
                        if pending:
                            emit_outproj_piece(*pending.popleft(), pidx)
                            pidx += 1
            for u in units[-SKEW:]:
                emit_back(*u)
                for _ in range(2):
                    if pending:
                        emit_outproj_piece(*pending.popleft(), pidx)
                        pidx += 1
            # keep the PE (and its HAM clock gate) busy while the final
            # head's rowsum-reciprocal chain resolves
            warm2 = y_ps.tile([128, QB], F32, tag="y", name="warm2")
            for wi in range(6):
                nc.tensor.matmul(
                    warm2[:], lhsT=ones[:], rhs=warm_sb[:],
                    start=(wi == 0), stop=(wi == 5),
                )
            for qb in ready:
                for qs in range(QB // 128):
                    for dc in range(D // QB):
                        pending.append((qb, qs, dc))
            while pending:
                emit_outproj_piece(*pending.popleft(), pidx, tail=True)
                pidx += 1

    nc.compile()
    return nc


_NC_CACHE: dict = {}


def _get_nc(causal: bool) -> bass.Bass:
    if causal not in _NC_CACHE:
        _NC_CACHE[causal] = build_nc(causal)
    return _NC_CACHE[causal]


def _e4m3(a):
    return np.clip(a, -240.0, 240.0).astype(ml_dtypes.float8_e4m3)


def _make_in_maps(x, attn_mask, Wq, bq, Wk, bk, Wv, bv, Wo, bo, causal):
    xT = np.ascontiguousarray(x.T).astype(np.float16)
    xT8 = _e4m3(x.T)
    maskT = None if causal else np.ascontiguousarray(attn_mask.T)
    in_maps = []
    for c in range(NCORES):
        sl = slice(c * CD, (c + 1) * CD)
        m = {
            "xT": np.ascontiguousarray(xT[:, :V8_STRIPE_START * QB]),
            "xT8": xT8,
            "wqT8": _e4m3(np.ascontiguousarray(Wq[sl, :].T) * W8SCALE),
            "wkT8": _e4m3(np.ascontiguousarray(Wk[sl, :].T) * W8SCALE),
            "wvT": np.ascontiguousarray(Wv[sl, :].T).astype(np.float16),
            "wvT8": _e4m3(np.ascontiguousarray(Wv[sl, :].T) * W8SCALE),
            "woT": np.ascontiguousarray(Wo[:, sl].T).astype(np.float16),
            "woT8": _e4m3(np.ascontiguousarray(Wo[:, sl].T) * W8SCALE),
            "bq": np.ascontiguousarray(bq[sl]),
            "bk": np.ascontiguousarray(bk[sl]),
        }
        if maskT is not None:
            m["maskT"] = maskT
        in_maps.append(m)
    return in_maps


def _is_causal(attn_mask) -> bool:
    if attn_mask.shape != (N, N):
        return False
    expected = np.where(
        np.tril(np.ones((N, N), dtype=bool)), np.float32(0.0), np.float32(NEG)
    )
    return bool(np.array_equal(attn_mask, expected))


def run_spmd(in_maps, causal, **kwargs):
    nc = _get_nc(causal)
    return run_bass_kernel_spmd(nc, in_maps, core_ids=list(range(NCORES)), **kwargs)


def kernel(x, attn_mask, Wq, bq, Wk, bk, Wv, bv, Wo, bo):
    causal = _is_causal(np.asarray(attn_mask))
    in_maps = _make_in_maps(
        np.asarray(x, np.float32), np.asarray(attn_mask, np.float32),
        np.asarray(Wq, np.float32), np.asarray(bq, np.float32),
        np.asarray(Wk, np.float32), np.asarray(bk, np.float32),
        np.asarray(Wv, np.float32), np.asarray(bv, np.float32),
        np.asarray(Wo, np.float32), np.asarray(bo, np.float32),
        causal,
    )
    res = run_spmd(in_maps, causal)
    # v's bias contribution folds exactly through the output projection
    # (attention rows sum to 1):  y += bo + Wo @ bv
    out = np.broadcast_to(
        (np.asarray(bo, np.float32)
         + np.asarray(Wo, np.float32) @ np.asarray(bv, np.float32)), (N, D)
    ).copy()
    for r in res.results:
        out += r["y"].astype(np.float32)
    return out


# revision 53
# speedup vs baseline: 1.1766x; 1.1766x over previous
"""Causal self-attention TRN2 kernel, tensor-parallel over heads on 8 NeuronCores.

Model (N=4096 tokens, D=2048, H=16 heads, HD=128):
    q = x @ Wq.T + bq ; k = x @ Wk.T + bk ; v = x @ Wv.T + bv   (per head)
    attn = softmax(q k^T / sqrt(HD) + causal_mask)
    y = concat_h(attn @ v) @ Wo.T + bo
Sharding: core c owns heads {2c, 2c+1} -> computes its QKV column slices,
attention for its heads, and a partial output projection
y_c = out_heads_c @ Wo[:, cols_c].T (+ bias/8).  Host sums the 8 partials.

Per-core kernel layout choices:
  * x is fed transposed (xT: D x N, fp16); an fp8 copy is cast on-device
    (DVE) for the Q/K projections, which run entirely in fp8e4 DoubleRow
    (256-deep contraction, 2x PE rate).  V stays fp16: fp8 V-projection
    noise passes straight through peaked attention rows (measured 3e-2).
  * q,k are produced directly transposed per head: qT/kT = (HD x N), fp16.
  * scores are computed transposed: sT[k,q] = kT_blk.T @ qT_blk, so the
    PV matmul needs no transposes at all: oT += v_blk.T @ exp(sT).
  * softmax skips the max-subtraction (scores are O(1); exp cannot
    overflow) -> row sums come from a ones-vector matmul on the PE, and
    1/rowsum is applied to oT (broadcast along partitions).
  * causality: key blocks entirely above the diagonal are skipped; blocks
    straddling the diagonal get -1e9 added via a precomputed triangular
    strip before the exp.
  * the attention pipeline works in key-block PAIRS: the two score matmuls
    of a pair share one 2-bank PSUM tile, so exp runs as a single 1024-wide
    ACT instruction (the ACT engine is the phase-2 co-bottleneck).
  * for query blocks >= QB8_START both exp(scores) and v are kept in fp8,
    so PV and the row-sum run as DoubleRow pair passes (2 key blocks per
    512-cycle pass).  Early rows (peaked attention) stay fp16.
  * v bias folds into the output bias exactly (attn rows sum to 1):
    y += (bo + Wo @ bv) / ncores  added on-device per core.
  * y partials leave the device as fp16 (host accumulates in fp32).
"""

from collections import deque
from contextlib import ExitStack

import numpy as np
import ml_dtypes

import concourse.bass as bass
import concourse.tile as tile
from concourse import bacc
from concourse import mybir
from concourse.bass_utils import run_bass_kernel_spmd
from concourse.masks import make_identity

N, D, H, HD = 4096, 2048, 16, 128
NCORES = 8
HPC = H // NCORES            # heads per core (2)
CD = HPC * HD                # per-core head-dim slice (256)
SCALE = 1.0 / float(np.sqrt(HD))
NEG = -1e9
W8SCALE = 16.0               # power-of-2 prescale keeping fp8 weights normal

QB = 512                     # query block (free dim of moving operands)
KB = 128                     # key block (partition dim of scores)
NQB = N // QB                # 8
KC8 = D // 256               # 256-deep contraction chunks (8)

F32 = mybir.dt.float32
F16 = mybir.dt.float16
F8 = mybir.dt.float8e4
DR = mybir.MatmulPerfMode.DoubleRow

# Query blocks >= QB8_START keep exp(scores) and v in fp8e4 so PV and the
# row-sum matmuls run in DoubleRow mode (2 key blocks per pass).  Early rows
# have peaked attention where quantization hurts; late rows average over many
# keys, so e4m3 noise washes out.
QB8_START = 1
# Query blocks >= OPROJ8_START run the output projection as a single fp8e4
# DoubleRow pass (256-deep: both heads at once).  The max error lives in the
# early rows (QK fp8 noise through peaked attention); fp8 outproj noise on
# later rows stays below it (verified against the reference inputs).
OPROJ8_START = 2
# V projection runs fp8e4 DoubleRow for token stripes >= 2: those v rows are
# only ever consumed through the (already fp8) v8 path by diffuse late query
# rows.  Stripes 0-1 keep the fp16 projection for the peaked early rows.
V8_STRIPE_START = 2


def build_nc(causal: bool = True) -> bass.Bass:
    nc = bacc.Bacc(None)

    # fp16 x is only needed for the fp16 V projection of the first
    # V8_STRIPE_START token stripes; everything else consumes the fp8 copy
    xT = nc.declare_dram_parameter(
        "xT", [D, V8_STRIPE_START * QB], F16, isOutput=False)
    xT8 = nc.declare_dram_parameter("xT8", [D, N], F8, isOutput=False)
    wqT8 = nc.declare_dram_parameter("wqT8", [D, CD], F8, isOutput=False)
    wkT8 = nc.declare_dram_parameter("wkT8", [D, CD], F8, isOutput=False)
    wvT = nc.declare_dram_parameter("wvT", [D, CD], F16, isOutput=False)
    wvT8 = nc.declare_dram_parameter("wvT8", [D, CD], F8, isOutput=False)
    woT = nc.declare_dram_parameter("woT", [CD, D], F16, isOutput=False)
    woT8 = nc.declare_dram_parameter("woT8", [CD, D], F8, isOutput=False)
    bq = nc.declare_dram_parameter("bq", [CD], F32, isOutput=False)
    bk = nc.declare_dram_parameter("bk", [CD], F32, isOutput=False)
    maskT = None
    if not causal:
        maskT = nc.declare_dram_parameter("maskT", [N, N], F32, isOutput=False)
    y = nc.declare_dram_parameter("y", [N, D], F16, isOutput=True)

    # fp16 v is only consumed by the fp16 PV path (query blocks < QB8_START),
    # which under causality only ever reads keys < QB8_START * QB.
    NV16 = QB8_START * QB if causal else N

    with tile.TileContext(nc) as tc, tc.tile_pool(name="persist", bufs=1) as persist:
        # ---------------- setup: weights, biases, constants -------------
        # Wo^T slice: (CD, D) -> per head (128, D); DMA'd later (scalar
        # queue, after the projection weights -- it is first needed at the
        # output projection, deep into phase 2).  fp16 for early query
        # blocks, fp8 (DoubleRow, both heads per pass) for the rest.
        wo_sb = persist.tile([128, HPC * D], F16, tag="wo")
        wo8_sb = persist.tile([128, HPC * D], F8, tag="wo8")
        # q/k biases: (CD,) -> (128, HPC), partition = dim within head
        bq_sb = persist.tile([128, HPC], F32, tag="bq")
        bk_sb = persist.tile([128, HPC], F32, tag="bk")
        nc.sync.dma_start(out=bq_sb[:], in_=bq[:].rearrange("(h p) -> p h", p=128))
        nc.sync.dma_start(out=bk_sb[:], in_=bk[:].rearrange("(h p) -> p h", p=128))
        # identity for PE transposes
        ident = persist.tile([128, 128], F16, tag="ident")
        # full ones matrix: row-sum matmul with this stationary operand
        # broadcasts the sum to all 128 output partitions at no extra cost
        ones = persist.tile([128, 128], F16, tag="ones")
        nc.vector.memset(ones[:], 1.0)
        # fp8 ones pair for DoubleRow row-sum matmuls (256-deep contraction)
        ones8 = persist.tile([128, 256], F8, tag="ones8")
        nc.vector.memset(ones8[:], 1.0)
        def emit_setup_selects():
            make_identity(nc, ident[:])

        # PE warm-up: dependency-free matmuls fill the DMA-startup window
        # and push the HAM clock gate to full rate before real work
        warm_sb = persist.tile([128, QB], F16, tag="warm")
        nc.vector.memset(warm_sb[:], 0.0)

        # Persistent activations: qT/kT per head (HD x N) fp16; v per head
        # stored (128, nblk*128 + hd) i.e. key-block-major with hd minor, in
        # fp16 (first NV16 keys only) and fp8 (all keys, feeds DR PV pairs).
        qT = [persist.tile([128, N], F16, tag=f"qT{h}", name=f"qT{h}")
              for h in range(HPC)]
        kT = [persist.tile([128, N], F16, tag=f"kT{h}", name=f"kT{h}")
              for h in range(HPC)]
        v_sb = [persist.tile([128, NV16], F16, tag=f"v{h}", name=f"v{h}")
                for h in range(HPC)]
        v8_sb = [persist.tile([128, N], F8, tag=f"v8{h}", name=f"v8{h}")
                 for h in range(HPC)]

        # ---------------- phase 1: QKV projections ----------------------
        # Q/K run in fp8e4 DoubleRow mode (256-deep contraction per pass,
        # 2x PE rate) on a device-cast fp8 copy of x; V stays fp16.
        with ExitStack() as p1:
            wproj = p1.enter_context(tc.tile_pool(name="wproj", bufs=1))
            xin = p1.enter_context(tc.tile_pool(name="xin", bufs=8))
            x8in = p1.enter_context(tc.tile_pool(name="x8in", bufs=8))
            vtpool = p1.enter_context(tc.tile_pool(name="vt", bufs=2))
            qkv_ps = p1.enter_context(tc.tile_pool(name="qkv_ps", bufs=6, space="PSUM"))
            tp_ps = p1.enter_context(tc.tile_pool(name="tp_ps", bufs=2, space="PSUM"))

            # Per-chunk weight tiles keep reader deps fine-grained: the first
            # matmul of chunk c only waits on chunk c's DMA, not the whole
            # weight array.  fp8 Q/K chunks [p, (j d)], contraction row
            # j*128+p; fp16 V chunks: two 128-deep subchunks [p, (g d)].
            wq8_c = [wproj.tile([128, 2 * CD], F8, tag=f"wq8_{c}", name=f"wq8_{c}")
                     for c in range(KC8)]
            wk8_c = [wproj.tile([128, 2 * CD], F8, tag=f"wk8_{c}", name=f"wk8_{c}")
                     for c in range(KC8)]
            wv8_c = [wproj.tile([128, 2 * CD], F8, tag=f"wv8_{c}", name=f"wv8_{c}")
                     for c in range(KC8)]
            wv_c = [wproj.tile([128, 2 * CD], F16, tag=f"wv_{c}", name=f"wv_{c}")
                    for c in range(KC8)]
            # weight DMAs go on the scalar queue so the sync queue is free to
            # carry half of the x stream from the very start
            for c in range(KC8):
                for w_sb, w_dram in ((wq8_c[c], wqT8), (wk8_c[c], wkT8),
                                     (wv8_c[c], wvT8)):
                    nc.scalar.dma_start(
                        out=w_sb[:].rearrange("p (j d) -> p j d", j=2),
                        in_=w_dram[c * 256:(c + 1) * 256, :].rearrange(
                            "(j p) d -> p j d", p=128),
                    )
                nc.scalar.dma_start(
                    out=wv_c[c][:].rearrange("p (g d) -> p g d", g=2),
                    in_=wvT[c * 256:(c + 1) * 256, :].rearrange(
                        "(g p) d -> p g d", p=128),
                )
            warm_ps = tp_ps.tile([128, QB], F32, tag="tp", name="warm_ps")
            for wi in range(20):
                nc.tensor.matmul(
                    warm_ps[:], lhsT=ones[:], rhs=warm_sb[:],
                    start=(wi == 0), stop=(wi == 19),
                )

            for nb in range(N // QB):  # 512-token stripes
                # psum tiles for qT/kT/vT of both heads
                pss = {}
                for nm in ("q", "k", "v"):
                    for h in range(HPC):
                        pss[nm, h] = qkv_ps.tile(
                            [128, QB], F32, tag="qkv", name=f"ps_{nm}{h}_{nb}"
                        )
                wnames = (((wq8_c, "q"), (wk8_c, "k"), (wv8_c, "v"))
                          if nb >= V8_STRIPE_START
                          else ((wq8_c, "q"), (wk8_c, "k")))
                for c in range(KC8):  # 256-deep contraction chunks
                    # alternate x chunks across two DMA queues for bandwidth
                    dma_eng = nc.gpsimd if c % 2 == 0 else nc.sync
                    x8t = x8in.tile([128, 2 * QB], F8, tag="x8",
                                    name=f"x8_{nb}_{c}")
                    dma_eng.dma_start(
                        out=x8t[:].rearrange("p (j q) -> p j q", j=2),
                        in_=xT8[c * 256:(c + 1) * 256,
                                nb * QB:(nb + 1) * QB].rearrange(
                            "(j p) q -> p j q", p=128),
                    )
                    x8_ap = x8t[:].rearrange("p (j q) -> p j q", j=2)
                    for w_c, nm in wnames:
                        w_ap = w_c[c][:].rearrange("p (j d) -> p j d", j=2)
                        for h in range(HPC):
                            nc.tensor.matmul(
                                pss[nm, h][:],
                                lhsT=w_ap[:, :, h * HD:(h + 1) * HD],
                                rhs=x8_ap,
                                start=(c == 0),
                                stop=(c == KC8 - 1),
                                perf_mode=DR,
                            )
                if nb < V8_STRIPE_START:
                    # fp16 V runs as a second sweep so the fp8 Q/K stream
                    # (whose x8 tiles land first) is never blocked on the
                    # bigger fp16 x transfers during the DMA ramp
                    for c in range(KC8):
                        xt = xin.tile([128, 2 * QB], F16, tag="xt",
                                      name=f"xt_{nb}_{c}")
                        dma_eng = nc.gpsimd if c % 2 == 0 else nc.sync
                        dma_eng.dma_start(
                            out=xt[:].rearrange("p (g q) -> p g q", g=2),
                            in_=xT[c * 256:(c + 1) * 256,
                                   nb * QB:(nb + 1) * QB].rearrange(
                                "(g p) q -> p g q", p=128),
                        )
                        for g in range(2):
                            for h in range(HPC):
                                nc.tensor.matmul(
                                    pss["v", h][:],
                                    lhsT=wv_c[c][:, g * CD + h * HD:
                                                 g * CD + (h + 1) * HD],
                                    rhs=xt[:, g * QB:(g + 1) * QB],
                                    start=(c == 0 and g == 0),
                                    stop=(c == KC8 - 1 and g == 1),
                                )
                if nb == 0:
                    # gpsimd setup ops, emitted after the first stripe's x DMAs
                    # so they don't block the queue head (ident is first needed
                    # by the v transposes just below)
                    emit_setup_selects()
                for nm, b_sb, dst in (("q", bq_sb, qT), ("k", bk_sb, kT)):
                    for h in range(HPC):
                        # 1/W8SCALE undoes the fp8 weight prescale
                        nc.scalar.activation(
                            out=dst[h][:, nb * QB:(nb + 1) * QB],
                            in_=pss[nm, h][:],
                            func=mybir.ActivationFunctionType.Identity,
                            bias=b_sb[:, h:h + 1],
                            scale=1.0 / W8SCALE,
                        )
                # v: evacuate vT (undoing the fp8 prescale for DR stripes),
                # then PE-transpose into (n, hd) layout, stored fp8 (all
                # keys) + fp16 (early keys)
                for h in range(HPC):
                    vt = vtpool.tile([128, QB], F16, tag="vt", name=f"vt_{nb}_{h}")
                    if nb >= V8_STRIPE_START:
                        nc.vector.tensor_scalar(
                            out=vt[:], in0=pss["v", h][:],
                            scalar1=1.0 / W8SCALE, scalar2=None,
                            op0=mybir.AluOpType.mult,
                        )
                    else:
                        nc.vector.tensor_copy(out=vt[:], in_=pss["v", h][:])
                    for s in range(QB // 128):
                        tp = tp_ps.tile([128, 128], F16, tag="tp",
                                        name=f"tp_{nb}_{h}_{s}")
                        nc.tensor.transpose(
                            tp[:], vt[:, s * 128:(s + 1) * 128], ident[:]
                        )
                        nblk = nb * (QB // 128) + s
                        nc.vector.tensor_copy(
                            out=v8_sb[h][:, nblk * 128:(nblk + 1) * 128],
                            in_=tp[:],
                        )
                        if nblk * 128 < NV16:
                            nc.vector.tensor_copy(
                                out=v_sb[h][:, nblk * 128:(nblk + 1) * 128],
                                in_=tp[:],
                            )

        # wo weight DMAs submit at the start of phase 2: they are first
        # needed ~30us later and would otherwise steal HBM bandwidth from
        # the x stream during the phase-1 ramp
        nc.scalar.dma_start(
            out=wo_sb[:].rearrange("p (h d) -> p h d", h=HPC),
            in_=woT[:].rearrange("(h p) d -> p h d", p=128),
        )
        nc.scalar.dma_start(
            out=wo8_sb[:].rearrange("p (h d) -> p h d", h=HPC),
            in_=woT8[:].rearrange("(h p) d -> p h d", p=128),
        )

        # ---------------- phase 2: attention + output projection --------
        with ExitStack() as p2:
            ptpool = p2.enter_context(tc.tile_pool(name="pt", bufs=8))
            otpool = p2.enter_context(tc.tile_pool(name="ot", bufs=6))
            ypool = p2.enter_context(tc.tile_pool(name="yout", bufs=12))
            small = p2.enter_context(tc.tile_pool(name="small", bufs=4))
            mtpool = p2.enter_context(tc.tile_pool(name="mt", bufs=4))
            # score-pair tiles: [128, 1024] fp32 = 2 PSUM banks each, so one
            # 1024-wide ACT exp covers both key blocks of a pair
            s_ps = p2.enter_context(tc.tile_pool(name="s_ps", bufs=2, space="PSUM"))
            o_ps = p2.enter_context(tc.tile_pool(name="o_ps", bufs=1, space="PSUM"))
            r_ps = p2.enter_context(tc.tile_pool(name="r_ps", bufs=1, space="PSUM"))
            y_ps = p2.enter_context(tc.tile_pool(name="y_ps", bufs=2, space="PSUM"))

            # Flat software-pipelined attention over key-block PAIRS:
            # scores/exp run SKEW pairs ahead of PV/rowsum, so the PE never
            # waits on the ACT exp latency (or the GpSimd diagonal-select
            # latency) -- including across head and query-block boundaries.
            # Output-projection pieces drip in between pairs to spread load.
            SKEW = 4
            units = []   # (qb, h, jp, npairs)
            for qb in range(NQB):
                npairs = (qb + 1) * (QB // KB) // 2 if causal else N // KB // 2
                for h in range(HPC):
                    for jp in range(npairs):
                        units.append((qb, h, jp, npairs))
            ready = []      # qblocks whose heads are normalized
            DELAY = 4       # pipeline pairs between normalize and outproj
            pending = deque()

            pts = {}
            o_psum = {}
            r_psum = {}
            oT_sb = {}

            def qoff_of(qb, nkb):
                # causal: columns q < off are fully masked for this key block;
                # skip them (exact -- their exp is 0)
                off = nkb * KB - qb * QB
                return max(0, off) if causal else 0

            def emit_front(qb, h, jp, npairs):
                fp8blk = causal and qb >= QB8_START
                sps = s_ps.tile([128, 2 * QB], F32, tag="s",
                                name=f"s_{qb}_{h}_{jp}")
                qoffs = []
                for j in range(2):
                    nkb = 2 * jp + j
                    qoff = qoff_of(qb, nkb)
                    qoffs.append(qoff)
                    w = QB - qoff
                    half = sps[:, j * QB:(j + 1) * QB]
                    nc.tensor.matmul(
                        half[:, qoff:],
                        lhsT=kT[h][:, nkb * KB:(nkb + 1) * KB],
                        rhs=qT[h][:, qb * QB + qoff:(qb + 1) * QB],
                        start=True,
                        stop=True,
                    )
                    if not causal:
                        mt = mtpool.tile([128, QB], F32, tag="mt",
                                         name=f"mt_{qb}_{h}_{jp}_{j}")
                        nc.sync.dma_start(
                            out=mt[:],
                            in_=maskT[nkb * KB:(nkb + 1) * KB,
                                      qb * QB:(qb + 1) * QB],
                        )
                        nc.vector.scalar_tensor_tensor(
                            out=half[:],
                            in0=mt[:],
                            scalar=1.0 / SCALE,
                            in1=half[:],
                            op0=mybir.AluOpType.mult,
                            op1=mybir.AluOpType.add,
                        )
                qoff0, qoff1 = qoffs
                if fp8blk:
                    pt2 = ptpool.tile([128, 2 * QB], F8, tag="pt8",
                                      name=f"pt8_{qb}_{h}_{jp}")
                    # one fused 1024-wide exp covers the whole pair; causal
                    # masking of diagonal blocks happens AFTER the exp as a
                    # triangular zero-fill on the fp8 tile (GpSimd, which is
                    # idle in phase 2 -- keeps ACT and DVE off this path)
                    nc.scalar.activation(
                        out=pt2[:], in_=sps[:],
                        func=mybir.ActivationFunctionType.Exp,
                        scale=SCALE,
                    )
                    for j in range(2):
                        off = (2 * jp + j) * KB - qb * QB
                        if causal and off >= 0:
                            nc.gpsimd.affine_select(
                                out=pt2[:, j * QB + qoff0:(j + 1) * QB],
                                in_=pt2[:, j * QB + qoff0:(j + 1) * QB],
                                compare_op=mybir.AluOpType.is_ge,
                                fill=0.0,
                                base=qoff0 - off,
                                pattern=[[1, QB - qoff0]],
                                channel_multiplier=-1,
                            )
                    return (pt2, qoff0)
                # fp16 path: two per-block exps into separate fp16 tiles
                res = []
                for j in range(2):
                    nkb = 2 * jp + j
                    qoff = qoffs[j]
                    w = QB - qoff
                    pt = ptpool.tile([128, QB], F16, tag="pt",
                                     name=f"pt_{qb}_{h}_{jp}_{j}")
                    nc.scalar.activation(
                        out=pt[:, :w], in_=sps[:, j * QB + qoff:(j + 1) * QB],
                        func=mybir.ActivationFunctionType.Exp,
                        scale=SCALE,
                    )
                    if causal and nkb * KB - qb * QB >= 0:
                        nc.gpsimd.affine_select(
                            out=pt[:, :w],
                            in_=pt[:, :w],
                            compare_op=mybir.AluOpType.is_ge,
                            fill=0.0,
                            base=0,
                            pattern=[[1, w]],
                            channel_multiplier=-1,
                        )
                    res.append(pt)
                return (res, None)

            def emit_outproj_piece(qb, qs, dc, pidx, tail=False):
                yps = y_ps.tile([128, QB], F32, tag="y",
                                name=f"y_{qb}_{qs}_{dc}")
                fp8piece = causal and qb >= OPROJ8_START
                if fp8piece:
                    # single DoubleRow pass: contraction over both heads'
                    # 128 oT dims at once (fp8 oT x fp8 Wo)
                    nc.tensor.matmul(
                        yps[:],
                        lhsT=oT_sb[qb][:].rearrange(
                            "p (j q) -> p j q", j=2)[:, :, qs * 128:(qs + 1) * 128],
                        rhs=wo8_sb[:].rearrange(
                            "p (j d) -> p j d", j=2)[:, :, dc * QB:(dc + 1) * QB],
                        start=True,
                        stop=True,
                        perf_mode=DR,
                    )
                else:
                    for h in range(HPC):
                        nc.tensor.matmul(
                            yps[:],
                            lhsT=oT_sb[qb, h][:, qs * 128:(qs + 1) * 128],
                            rhs=wo_sb[:, h * D + dc * QB: h * D + (dc + 1) * QB],
                            start=(h == 0),
                            stop=(h == HPC - 1),
                        )
                ysb = ypool.tile([128, QB], F16, tag="ysb",
                                 name=f"ys_{qb}_{qs}_{dc}")
                # psum evacuation on DVE (output bias is added on the host;
                # fp8 pieces also undo the Wo prescale); during the final
                # flush the exp stream is done, so ACT shares the evacuation
                # load and the y DMAs fan out over two queues
                scale = 1.0 / W8SCALE if fp8piece else 1.0
                if tail and pidx % 2 == 1:
                    nc.scalar.activation(
                        out=ysb[:], in_=yps[:],
                        func=mybir.ActivationFunctionType.Identity,
                        scale=scale,
                    )
                elif fp8piece:
                    nc.vector.tensor_scalar(
                        out=ysb[:], in0=yps[:], scalar1=scale,
                        scalar2=None, op0=mybir.AluOpType.mult,
                    )
                else:
                    nc.vector.tensor_copy(out=ysb[:], in_=yps[:])
                row0 = qb * QB + qs * 128
                dma_eng = nc.gpsimd if (tail and pidx % 2 == 1) else nc.sync
                dma_eng.dma_start(
                    out=y[row0:row0 + 128, dc * QB:(dc + 1) * QB], in_=ysb[:]
                )

            def emit_back(qb, h, jp, npairs):
                fp8blk = causal and qb >= QB8_START
                if jp == 0:
                    o_psum[qb, h] = o_ps.tile([128, QB], F32, tag="o",
                                              name=f"o_{qb}_{h}")
                    r_psum[qb, h] = r_ps.tile([128, QB], F32, tag="r",
                                              name=f"r_{qb}_{h}")
                pt, qoff0 = pts.pop((qb, h, jp))
                if fp8blk:
                    # fp8 DoubleRow pair passes: PV and rowsum cover both key
                    # blocks in one 256-deep pass each.  The rowsum goes
                    # first: at head boundaries the o bank may still be held
                    # by the previous head's normalize, and the r-bank pass
                    # hides part of that wait.
                    pt2_ap = pt[:].rearrange("p (j q) -> p j q", j=2)
                    nc.tensor.matmul(
                        r_psum[qb, h][:, qoff0:],
                        lhsT=ones8[:].rearrange("p (j c) -> p j c", j=2),
                        rhs=pt2_ap[:, :, qoff0:],
                        start=(jp == 0),
                        stop=(jp == npairs - 1),
                        perf_mode=DR,
                    )
                    nc.tensor.matmul(
                        o_psum[qb, h][:, qoff0:],
                        lhsT=v8_sb[h][:, jp * 256:(jp + 1) * 256].rearrange(
                            "p (j d) -> p j d", j=2),
                        rhs=pt2_ap[:, :, qoff0:],
                        start=(jp == 0),
                        stop=(jp == npairs - 1),
                        perf_mode=DR,
                    )
                else:
                    for j in range(2):
                        nkb = 2 * jp + j
                        qoff = qoff_of(qb, nkb)
                        w = QB - qoff
                        nc.tensor.matmul(
                            r_psum[qb, h][:, qoff:],
                            lhsT=ones[:],
                            rhs=pt[j][:, :w],
                            start=(nkb == 0),
                            stop=(nkb == 2 * npairs - 1),
                        )
                        nc.tensor.matmul(
                            o_psum[qb, h][:, qoff:],
                            lhsT=v_sb[h][:, nkb * KB:(nkb + 1) * KB],
                            rhs=pt[j][:, :w],
                            start=(nkb == 0),
                            stop=(nkb == 2 * npairs - 1),
                        )
                if jp == npairs - 1:
                    # fast approx reciprocal straight from PSUM (frees the r
                    # bank in one op), then one DVE multiply normalizes oT
                    rbc = small.tile([128, QB], F32, tag="rbc",
                                     name=f"rb_{qb}_{h}")
                    nc.vector.reciprocal_approx_fast(
                        out=rbc[:], in_=r_psum.pop((qb, h))[:])
                    if causal and qb >= OPROJ8_START:
                        # both heads' normalized oT land in one fp8 tile so
                        # the output projection runs as a single DoubleRow
                        # pass (256-deep contraction: h0+h1)
                        if h == 0:
                            oT_sb[qb] = otpool.tile(
                                [128, 2 * QB], F8, tag="ot8", name=f"ot8_{qb}")
                        nc.vector.tensor_mul(
                            oT_sb[qb][:, h * QB:(h + 1) * QB],
                            o_psum.pop((qb, h))[:], rbc[:])
                    else:
                        ot = otpool.tile([128, QB], F16, tag="ot",
                                         name=f"ot_{qb}_{h}")
                        nc.vector.tensor_mul(
                            ot[:], o_psum.pop((qb, h))[:], rbc[:])
                        oT_sb[qb, h] = ot
                    if h == HPC - 1:
                        ready.append(qb)

            ready_at = {}
            pidx = 0
            for i, u in enumerate(units):
                pts[u[:3]] = emit_front(*u)
                if i >= SKEW:
                    n_ready = len(ready)
                    emit_back(*units[i - SKEW])
                    if len(ready) > n_ready:
                        ready_at[ready[-1]] = i
                while ready and i - ready_at[ready[0]] >= DELAY:
                    qb = ready.pop(0)
                    for qs in range(QB // 128):
                        for dc in range(D // QB):
                            pending.append((qb, qs, dc))
                # drip outproj pieces between pairs to spread the load
                for _ in range(2):
                    if pending:
                        emit_outproj_piece(*pending.popleft(), pidx)
                        pidx += 1
            for u in units[-SKEW:]:
                emit_back(*u)
                for _ in range(2):
                    if pending:
                        emit_outproj_piece(*pending.popleft(), pidx)
                        pidx += 1
            # keep the PE (and its HAM clock gate) busy while the final
            # head's rowsum-reciprocal chain resolves
            warm2 = y_ps.tile([128, QB], F32, tag="y", name="warm2")
            for wi in range(6):
                nc.tensor.matmul(
                    warm2[:], lhsT=ones[:], rhs=warm_sb[:],
                    start=(wi == 0), stop=(wi == 5),
                )
            for qb in ready:
                for qs in range(QB // 128):
                    for dc in range(D // QB):
                        pending.append((qb, qs, dc))
            while pending:
                emit_outproj_piece(*pending.popleft(), pidx, tail=True)
                pidx += 1

    nc.compile()
    return nc


_NC_CACHE: dict = {}


def _get_nc(causal: bool) -> bass.Bass:
    if causal not in _NC_CACHE:
        _NC_CACHE[causal] = build_nc(causal)
    return _NC_CACHE[causal]


def _e4m3(a):
    return np.clip(a, -240.0, 240.0).astype(ml_dtypes.float8_e4m3)


def _make_in_maps(x, attn_mask, Wq, bq, Wk, bk, Wv, bv, Wo, bo, causal):
    xT = np.ascontiguousarray(x.T).astype(np.float16)
    xT8 = _e4m3(x.T)
    maskT = None if causal else np.ascontiguousarray(attn_mask.T)
    in_maps = []
    for c in range(NCORES):
        sl = slice(c * CD, (c + 1) * CD)
        m = {
            "xT": np.ascontiguousarray(xT[:, :V8_STRIPE_START * QB]),
            "xT8": xT8,
            "wqT8": _e4m3(np.ascontiguousarray(Wq[sl, :].T) * W8SCALE),
            "wkT8": _e4m3(np.ascontiguousarray(Wk[sl, :].T) * W8SCALE),
            "wvT": np.ascontiguousarray(Wv[sl, :].T).astype(np.float16),
            "wvT8": _e4m3(np.ascontiguousarray(Wv[sl, :].T) * W8SCALE),
            "woT": np.ascontiguousarray(Wo[:, sl].T).astype(np.float16),
            "woT8": _e4m3(np.ascontiguousarray(Wo[:, sl].T) * W8SCALE),
            "bq": np.ascontiguousarray(bq[sl]),
            "bk": np.ascontiguousarray(bk[sl]),
        }
        if maskT is not None:
            m["maskT"] = maskT
        in_maps.append(m)
    return in_maps


def _is_causal(attn_mask) -> bool:
    if attn_mask.shape != (N, N):
        return False
    expected = np.where(
        np.tril(np.ones((N, N), dtype=bool)), np.float32(0.0), np.float32(NEG)
    )
    return bool(np.array_equal(attn_mask, expected))


def run_spmd(in_maps, causal, **kwargs):
    nc = _get_nc(causal)
    return run_bass_kernel_spmd(nc, in_maps, core_ids=list(range(NCORES)), **kwargs)


def kernel(x, attn_mask, Wq, bq, Wk, bk, Wv, bv, Wo, bo):
    causal = _is_causal(np.asarray(attn_mask))
    in_maps = _make_in_maps(
        np.asarray(x, np.float32), np.asarray(attn_mask, np.float32),
        np.asarray(Wq, np.float32), np.asarray(bq, np.float32),
        np.asarray(Wk, np.float32), np.asarray(bk, np.float32),
        np.asarray(Wv, np.float32), np.asarray(bv, np.float32),
        np.asarray(Wo, np.float32), np.asarray(bo, np.float32),
        causal,
    )
    res = run_spmd(in_maps, causal)
    # v's bias contribution folds exactly through the output projection
    # (attention rows sum to 1):  y += bo + Wo @ bv
    out = np.broadcast_to(
        (np.asarray(bo, np.float32)
         + np.asarray(Wo, np.float32) @ np.asarray(bv, np.float32)), (N, D)
    ).copy()
    for r in res.results:
        out += r["y"].astype(np.float32)
    return out


# revision 60
# speedup vs baseline: 1.2263x; 1.0422x over previous
"""Causal self-attention TRN2 kernel, tensor-parallel over heads on 8 NeuronCores.

Model (N=4096 tokens, D=2048, H=16 heads, HD=128):
    q = x @ Wq.T + bq ; k = x @ Wk.T + bk ; v = x @ Wv.T + bv   (per head)
    attn = softmax(q k^T / sqrt(HD) + causal_mask)
    y = concat_h(attn @ v) @ Wo.T + bo
Sharding: core c owns heads {2c, 2c+1} -> computes its QKV column slices,
attention for its heads, and a partial output projection
y_c = out_heads_c @ Wo[:, cols_c].T (+ bias/8).  Host sums the 8 partials.

Per-core kernel layout choices:
  * x is fed transposed (xT: D x N, fp16); an fp8 copy is cast on-device
    (DVE) for the Q/K projections, which run entirely in fp8e4 DoubleRow
    (256-deep contraction, 2x PE rate).  V stays fp16: fp8 V-projection
    noise passes straight through peaked attention rows (measured 3e-2).
  * q,k are produced directly transposed per head: qT/kT = (HD x N), fp16.
  * scores are computed transposed: sT[k,q] = kT_blk.T @ qT_blk, so the
    PV matmul needs no transposes at all: oT += v_blk.T @ exp(sT).
  * softmax skips the max-subtraction (scores are O(1); exp cannot
    overflow) -> row sums come from a ones-vector matmul on the PE, and
    1/rowsum is applied to oT (broadcast along partitions).
  * causality: key blocks entirely above the diagonal are skipped; blocks
    straddling the diagonal get -1e9 added via a precomputed triangular
    strip before the exp.
  * the attention pipeline works in key-block PAIRS: the two score matmuls
    of a pair share one 2-bank PSUM tile, so exp runs as a single 1024-wide
    ACT instruction (the ACT engine is the phase-2 co-bottleneck).
  * for query blocks >= QB8_START both exp(scores) and v are kept in fp8,
    so PV and the row-sum run as DoubleRow pair passes (2 key blocks per
    512-cycle pass).  Early rows (peaked attention) stay fp16.
  * v bias folds into the output bias exactly (attn rows sum to 1):
    y += (bo + Wo @ bv) / ncores  added on-device per core.
  * y partials leave the device as fp16 (host accumulates in fp32).
"""

from collections import deque
from contextlib import ExitStack

import numpy as np
import ml_dtypes

import concourse.bass as bass
import concourse.tile as tile
from concourse import bacc
from concourse import mybir
from concourse.bass_utils import run_bass_kernel_spmd
from concourse.masks import make_identity

N, D, H, HD = 4096, 2048, 16, 128
NCORES = 8
HPC = H // NCORES            # heads per core (2)
CD = HPC * HD                # per-core head-dim slice (256)
SCALE = 1.0 / float(np.sqrt(HD))
NEG = -1e9
W8SCALE = 16.0               # power-of-2 prescale keeping fp8 weights normal

QB = 512                     # query block (free dim of moving operands)
KB = 128                     # key block (partition dim of scores)
NQB = N // QB                # 8
KC8 = D // 256               # 256-deep contraction chunks (8)

F32 = mybir.dt.float32
F16 = mybir.dt.float16
F8 = mybir.dt.float8e4
DR = mybir.MatmulPerfMode.DoubleRow

# Query blocks >= QB8_START keep exp(scores) and v in fp8e4 so PV and the
# row-sum matmuls run in DoubleRow mode (2 key blocks per pass).  Early rows
# have peaked attention where quantization hurts; late rows average over many
# keys, so e4m3 noise washes out.
QB8_START = 1
# Query blocks >= OPROJ8_START run the output projection as a single fp8e4
# DoubleRow pass (256-deep: both heads at once).  The max error lives in the
# early rows (QK fp8 noise through peaked attention); fp8 outproj noise on
# later rows stays below it (verified against the reference inputs).
OPROJ8_START = 2
# V projection runs fp8e4 DoubleRow for token stripes >= 1: those v rows are
# only ever consumed through the (already fp8) v8 path by diffuse late query
# rows.  Stripe 0 keeps the fp16 projection for the peaked early rows.
V8_STRIPE_START = 1


def build_nc(causal: bool = True) -> bass.Bass:
    nc = bacc.Bacc(None)

    # All bulk inputs are host-packed into the exact SBUF tile layouts, so
    # every DMA below is a flat contiguous row-slice (one max-size packet
    # per partition -- no strided descriptors).
    # fp16 x is only needed for the fp16 V projection of the first
    # V8_STRIPE_START token stripes; everything else consumes the fp8 copy.
    xT = nc.declare_dram_parameter(
        "xT", [V8_STRIPE_START * KC8 * 128, 2 * QB], F16, isOutput=False)
    xT8 = nc.declare_dram_parameter(
        "xT8", [KC8 * NQB * 128, 2 * QB], F8, isOutput=False)
    wqT8 = nc.declare_dram_parameter("wqT8", [KC8 * 128, 2 * CD], F8, isOutput=False)
    wkT8 = nc.declare_dram_parameter("wkT8", [KC8 * 128, 2 * CD], F8, isOutput=False)
    wvT = nc.declare_dram_parameter("wvT", [KC8 * 128, 2 * CD], F16, isOutput=False)
    wvT8 = nc.declare_dram_parameter("wvT8", [KC8 * 128, 2 * CD], F8, isOutput=False)
    woT = nc.declare_dram_parameter("woT", [128, HPC * D], F16, isOutput=False)
    woT8 = nc.declare_dram_parameter("woT8", [128, HPC * D], F8, isOutput=False)
    bq = nc.declare_dram_parameter("bq", [CD], F32, isOutput=False)
    bk = nc.declare_dram_parameter("bk", [CD], F32, isOutput=False)
    maskT = None
    if not causal:
        maskT = nc.declare_dram_parameter("maskT", [N, N], F32, isOutput=False)
    y = nc.declare_dram_parameter("y", [N, D], F16, isOutput=True)

    # fp16 v is only consumed by the fp16 PV path (query blocks < QB8_START),
    # which under causality only ever reads keys < QB8_START * QB.
    NV16 = QB8_START * QB if causal else N

    with tile.TileContext(nc) as tc, tc.tile_pool(name="persist", bufs=1) as persist:
        # ---------------- setup: weights, biases, constants -------------
        # Wo^T slice: (CD, D) -> per head (128, D); DMA'd later (scalar
        # queue, after the projection weights -- it is first needed at the
        # output projection, deep into phase 2).  fp16 for early query
        # blocks, fp8 (DoubleRow, both heads per pass) for the rest.
        wo_sb = persist.tile([128, HPC * D], F16, tag="wo")
        wo8_sb = persist.tile([128, HPC * D], F8, tag="wo8")
        # q/k biases: (CD,) -> (128, HPC), partition = dim within head
        bq_sb = persist.tile([128, HPC], F32, tag="bq")
        bk_sb = persist.tile([128, HPC], F32, tag="bk")
        nc.sync.dma_start(out=bq_sb[:], in_=bq[:].rearrange("(h p) -> p h", p=128))
        nc.sync.dma_start(out=bk_sb[:], in_=bk[:].rearrange("(h p) -> p h", p=128))
        # identity for PE transposes
        ident = persist.tile([128, 128], F16, tag="ident")
        # full ones matrix: row-sum matmul with this stationary operand
        # broadcasts the sum to all 128 output partitions at no extra cost
        ones = persist.tile([128, 128], F16, tag="ones")
        nc.vector.memset(ones[:], 1.0)
        # fp8 ones pair for DoubleRow row-sum matmuls (256-deep contraction)
        ones8 = persist.tile([128, 256], F8, tag="ones8")
        nc.vector.memset(ones8[:], 1.0)
        def emit_setup_selects():
            make_identity(nc, ident[:])

        # PE warm-up: dependency-free matmuls fill the DMA-startup window
        # and push the HAM clock gate to full rate before real work
        warm_sb = persist.tile([128, QB], F16, tag="warm")
        nc.vector.memset(warm_sb[:], 0.0)

        # Persistent activations: qT/kT per head (HD x N) fp16; v per head
        # stored (128, nblk*128 + hd) i.e. key-block-major with hd minor, in
        # fp16 (first NV16 keys only) and fp8 (all keys, feeds DR PV pairs).
        qT = [persist.tile([128, N], F16, tag=f"qT{h}", name=f"qT{h}")
              for h in range(HPC)]
        kT = [persist.tile([128, N], F16, tag=f"kT{h}", name=f"kT{h}")
              for h in range(HPC)]
        v_sb = [persist.tile([128, NV16], F16, tag=f"v{h}", name=f"v{h}")
                for h in range(HPC)]
        v8_sb = [persist.tile([128, N], F8, tag=f"v8{h}", name=f"v8{h}")
                 for h in range(HPC)]

        # ---------------- phase 1: QKV projections ----------------------
        # Q/K run in fp8e4 DoubleRow mode (256-deep contraction per pass,
        # 2x PE rate) on a device-cast fp8 copy of x; V stays fp16.
        with ExitStack() as p1:
            wproj = p1.enter_context(tc.tile_pool(name="wproj", bufs=1))
            xin = p1.enter_context(tc.tile_pool(name="xin", bufs=8))
            x8in = p1.enter_context(tc.tile_pool(name="x8in", bufs=8))
            vtpool = p1.enter_context(tc.tile_pool(name="vt", bufs=2))
            qkv_ps = p1.enter_context(tc.tile_pool(name="qkv_ps", bufs=6, space="PSUM"))
            tp_ps = p1.enter_context(tc.tile_pool(name="tp_ps", bufs=2, space="PSUM"))

            # Per-chunk weight tiles keep reader deps fine-grained: the first
            # matmul of chunk c only waits on chunk c's DMA, not the whole
            # weight array.  fp8 Q/K chunks [p, (j d)], contraction row
            # j*128+p; fp16 V chunks: two 128-deep subchunks [p, (g d)].
            wq8_c = [wproj.tile([128, 2 * CD], F8, tag=f"wq8_{c}", name=f"wq8_{c}")
                     for c in range(KC8)]
            wk8_c = [wproj.tile([128, 2 * CD], F8, tag=f"wk8_{c}", name=f"wk8_{c}")
                     for c in range(KC8)]
            wv8_c = [wproj.tile([128, 2 * CD], F8, tag=f"wv8_{c}", name=f"wv8_{c}")
                     for c in range(KC8)]
            wv_c = [wproj.tile([128, 2 * CD], F16, tag=f"wv_{c}", name=f"wv_{c}")
                    for c in range(KC8)]
            # weight DMAs go on the scalar queue so the sync queue is free to
            # carry half of the x stream from the very start
            for c in range(KC8):
                for w_sb, w_dram in ((wq8_c[c], wqT8), (wk8_c[c], wkT8),
                                     (wv8_c[c], wvT8), (wv_c[c], wvT)):
                    nc.scalar.dma_start(
                        out=w_sb[:],
                        in_=w_dram[c * 128:(c + 1) * 128, :],
                    )
            warm_ps = tp_ps.tile([128, QB], F32, tag="tp", name="warm_ps")
            for wi in range(20):
                nc.tensor.matmul(
                    warm_ps[:], lhsT=ones[:], rhs=warm_sb[:],
                    start=(wi == 0), stop=(wi == 19),
                )

            for nb in range(N // QB):  # 512-token stripes
                # psum tiles for qT/kT/vT of both heads
                pss = {}
                for nm in ("q", "k", "v"):
                    for h in range(HPC):
                        pss[nm, h] = qkv_ps.tile(
                            [128, QB], F32, tag="qkv", name=f"ps_{nm}{h}_{nb}"
                        )
                wnames = (((wq8_c, "q"), (wk8_c, "k"), (wv8_c, "v"))
                          if nb >= V8_STRIPE_START
                          else ((wq8_c, "q"), (wk8_c, "k")))
                for c in range(KC8):  # 256-deep contraction chunks
                    # alternate x chunks across two DMA queues for bandwidth
                    dma_eng = nc.gpsimd if c % 2 == 0 else nc.sync
                    x8t = x8in.tile([128, 2 * QB], F8, tag="x8",
                                    name=f"x8_{nb}_{c}")
                    r0 = (c * NQB + nb) * 128
                    dma_eng.dma_start(out=x8t[:], in_=xT8[r0:r0 + 128, :])
                    x8_ap = x8t[:].rearrange("p (j q) -> p j q", j=2)
                    for w_c, nm in wnames:
                        w_ap = w_c[c][:].rearrange("p (j d) -> p j d", j=2)
                        for h in range(HPC):
                            nc.tensor.matmul(
                                pss[nm, h][:],
                                lhsT=w_ap[:, :, h * HD:(h + 1) * HD],
                                rhs=x8_ap,
                                start=(c == 0),
                                stop=(c == KC8 - 1),
                                perf_mode=DR,
                            )
                if nb < V8_STRIPE_START:
                    # fp16 V runs as a second sweep so the fp8 Q/K stream
                    # (whose x8 tiles land first) is never blocked on the
                    # bigger fp16 x transfers during the DMA ramp
                    for c in range(KC8):
                        xt = xin.tile([128, 2 * QB], F16, tag="xt",
                                      name=f"xt_{nb}_{c}")
                        dma_eng = nc.gpsimd if c % 2 == 0 else nc.sync
                        r0 = (nb * KC8 + c) * 128
                        dma_eng.dma_start(out=xt[:], in_=xT[r0:r0 + 128, :])
                        for g in range(2):
                            for h in range(HPC):
                                nc.tensor.matmul(
                                    pss["v", h][:],
                                    lhsT=wv_c[c][:, g * CD + h * HD:
                                                 g * CD + (h + 1) * HD],
                                    rhs=xt[:, g * QB:(g + 1) * QB],
                                    start=(c == 0 and g == 0),
                                    stop=(c == KC8 - 1 and g == 1),
                                )
                if nb == 0:
                    # gpsimd setup ops, emitted after the first stripe's x DMAs
                    # so they don't block the queue head (ident is first needed
                    # by the v transposes just below)
                    emit_setup_selects()
                for nm, b_sb, dst in (("q", bq_sb, qT), ("k", bk_sb, kT)):
                    for h in range(HPC):
                        # 1/W8SCALE undoes the fp8 weight prescale
                        nc.scalar.activation(
                            out=dst[h][:, nb * QB:(nb + 1) * QB],
                            in_=pss[nm, h][:],
                            func=mybir.ActivationFunctionType.Identity,
                            bias=b_sb[:, h:h + 1],
                            scale=1.0 / W8SCALE,
                        )
                # v: evacuate vT (undoing the fp8 prescale for DR stripes),
                # then PE-transpose into (n, hd) layout, stored fp8 (all
                # keys) + fp16 (early keys)
                for h in range(HPC):
                    vt = vtpool.tile([128, QB], F16, tag="vt", name=f"vt_{nb}_{h}")
                    if nb >= V8_STRIPE_START:
                        nc.vector.tensor_scalar(
                            out=vt[:], in0=pss["v", h][:],
                            scalar1=1.0 / W8SCALE, scalar2=None,
                            op0=mybir.AluOpType.mult,
                        )
                    else:
                        nc.vector.tensor_copy(out=vt[:], in_=pss["v", h][:])
                    for s in range(QB // 128):
                        tp = tp_ps.tile([128, 128], F16, tag="tp",
                                        name=f"tp_{nb}_{h}_{s}")
                        nc.tensor.transpose(
                            tp[:], vt[:, s * 128:(s + 1) * 128], ident[:]
                        )
                        nblk = nb * (QB // 128) + s
                        nc.vector.tensor_copy(
                            out=v8_sb[h][:, nblk * 128:(nblk + 1) * 128],
                            in_=tp[:],
                        )
                        if nblk * 128 < NV16:
                            nc.vector.tensor_copy(
                                out=v_sb[h][:, nblk * 128:(nblk + 1) * 128],
                                in_=tp[:],
                            )

        # wo weight DMAs submit at the start of phase 2: they are first
        # needed ~30us later and would otherwise steal HBM bandwidth from
        # the x stream during the phase-1 ramp
        nc.scalar.dma_start(out=wo_sb[:], in_=woT[:])
        nc.scalar.dma_start(out=wo8_sb[:], in_=woT8[:])

        # ---------------- phase 2: attention + output projection --------
        with ExitStack() as p2:
            ptpool = p2.enter_context(tc.tile_pool(name="pt", bufs=8))
            otpool = p2.enter_context(tc.tile_pool(name="ot", bufs=6))
            ypool = p2.enter_context(tc.tile_pool(name="yout", bufs=12))
            small = p2.enter_context(tc.tile_pool(name="small", bufs=4))
            mtpool = p2.enter_context(tc.tile_pool(name="mt", bufs=4))
            # score-pair tiles: [128, 1024] fp32 = 2 PSUM banks each, so one
            # 1024-wide ACT exp covers both key blocks of a pair
            s_ps = p2.enter_context(tc.tile_pool(name="s_ps", bufs=2, space="PSUM"))
            o_ps = p2.enter_context(tc.tile_pool(name="o_ps", bufs=1, space="PSUM"))
            r_ps = p2.enter_context(tc.tile_pool(name="r_ps", bufs=1, space="PSUM"))
            y_ps = p2.enter_context(tc.tile_pool(name="y_ps", bufs=2, space="PSUM"))

            # Flat software-pipelined attention over key-block PAIRS:
            # scores/exp run SKEW pairs ahead of PV/rowsum, so the PE never
            # waits on the ACT exp latency (or the GpSimd diagonal-select
            # latency) -- including across head and query-block boundaries.
            # Output-projection pieces drip in between pairs to spread load.
            SKEW = 4
            units = []   # (qb, h, jp, npairs)
            for qb in range(NQB):
                npairs = (qb + 1) * (QB // KB) // 2 if causal else N // KB // 2
                for h in range(HPC):
                    for jp in range(npairs):
                        units.append((qb, h, jp, npairs))
            ready = []      # qblocks whose heads are normalized
            DELAY = 4       # pipeline pairs between normalize and outproj
            pending = deque()

            pts = {}
            o_psum = {}
            r_psum = {}
            oT_sb = {}

            def qoff_of(qb, nkb):
                # causal: columns q < off are fully masked for this key block;
                # skip them (exact -- their exp is 0)
                off = nkb * KB - qb * QB
                return max(0, off) if causal else 0

            def emit_front(qb, h, jp, npairs):
                fp8blk = causal and qb >= QB8_START
                sps = s_ps.tile([128, 2 * QB], F32, tag="s",
                                name=f"s_{qb}_{h}_{jp}")
                qoffs = []
                for j in range(2):
                    nkb = 2 * jp + j
                    qoff = qoff_of(qb, nkb)
                    qoffs.append(qoff)
                    w = QB - qoff
                    half = sps[:, j * QB:(j + 1) * QB]
                    nc.tensor.matmul(
                        half[:, qoff:],
                        lhsT=kT[h][:, nkb * KB:(nkb + 1) * KB],
                        rhs=qT[h][:, qb * QB + qoff:(qb + 1) * QB],
                        start=True,
                        stop=True,
                    )
                    if not causal:
                        mt = mtpool.tile([128, QB], F32, tag="mt",
                                         name=f"mt_{qb}_{h}_{jp}_{j}")
                        nc.sync.dma_start(
                            out=mt[:],
                            in_=maskT[nkb * KB:(nkb + 1) * KB,
                                      qb * QB:(qb + 1) * QB],
                        )
                        nc.vector.scalar_tensor_tensor(
                            out=half[:],
                            in0=mt[:],
                            scalar=1.0 / SCALE,
                            in1=half[:],
                            op0=mybir.AluOpType.mult,
                            op1=mybir.AluOpType.add,
                        )
                qoff0, qoff1 = qoffs
                if fp8blk:
                    pt2 = ptpool.tile([128, 2 * QB], F8, tag="pt8",
                                      name=f"pt8_{qb}_{h}_{jp}")
                    # one fused 1024-wide exp covers the whole pair; causal
                    # masking of diagonal blocks happens AFTER the exp as a
                    # triangular zero-fill on the fp8 tile (GpSimd, which is
                    # idle in phase 2 -- keeps ACT and DVE off this path)
                    nc.scalar.activation(
                        out=pt2[:], in_=sps[:],
                        func=mybir.ActivationFunctionType.Exp,
                        scale=SCALE,
                    )
                    for j in range(2):
                        off = (2 * jp + j) * KB - qb * QB
                        if causal and off >= 0:
                            nc.gpsimd.affine_select(
                                out=pt2[:, j * QB + qoff0:(j + 1) * QB],
                                in_=pt2[:, j * QB + qoff0:(j + 1) * QB],
                                compare_op=mybir.AluOpType.is_ge,
                                fill=0.0,
                                base=qoff0 - off,
                                pattern=[[1, QB - qoff0]],
                                channel_multiplier=-1,
                            )
                    return (pt2, qoff0)
                # fp16 path: two per-block exps into separate fp16 tiles
                res = []
                for j in range(2):
                    nkb = 2 * jp + j
                    qoff = qoffs[j]
                    w = QB - qoff
                    pt = ptpool.tile([128, QB], F16, tag="pt",
                                     name=f"pt_{qb}_{h}_{jp}_{j}")
                    nc.scalar.activation(
                        out=pt[:, :w], in_=sps[:, j * QB + qoff:(j + 1) * QB],
                        func=mybir.ActivationFunctionType.Exp,
                        scale=SCALE,
                    )
                    if causal and nkb * KB - qb * QB >= 0:
                        nc.gpsimd.affine_select(
                            out=pt[:, :w],
                            in_=pt[:, :w],
                            compare_op=mybir.AluOpType.is_ge,
                            fill=0.0,
                            base=0,
                            pattern=[[1, w]],
                            channel_multiplier=-1,
                        )
                    res.append(pt)
                return (res, None)

            def emit_outproj_piece(qb, qs, dc, pidx, tail=False):
                yps = y_ps.tile([128, QB], F32, tag="y",
                                name=f"y_{qb}_{qs}_{dc}")
                fp8piece = causal and qb >= OPROJ8_START
                if fp8piece:
                    # single DoubleRow pass: contraction over both heads'
                    # 128 oT dims at once (fp8 oT x fp8 Wo)
                    nc.tensor.matmul(
                        yps[:],
                        lhsT=oT_sb[qb][:].rearrange(
                            "p (j q) -> p j q", j=2)[:, :, qs * 128:(qs + 1) * 128],
                        rhs=wo8_sb[:].rearrange(
                            "p (j d) -> p j d", j=2)[:, :, dc * QB:(dc + 1) * QB],
                        start=True,
                        stop=True,
                        perf_mode=DR,
                    )
                else:
                    for h in range(HPC):
                        nc.tensor.matmul(
                            yps[:],
                            lhsT=oT_sb[qb, h][:, qs * 128:(qs + 1) * 128],
                            rhs=wo_sb[:, h * D + dc * QB: h * D + (dc + 1) * QB],
                            start=(h == 0),
                            stop=(h == HPC - 1),
                        )
                ysb = ypool.tile([128, QB], F16, tag="ysb",
                                 name=f"ys_{qb}_{qs}_{dc}")
                # psum evacuation on DVE (output bias is added on the host;
                # fp8 pieces also undo the Wo prescale); during the final
                # flush the exp stream is done, so ACT shares the evacuation
                # load and the y DMAs fan out over two queues
                scale = 1.0 / W8SCALE if fp8piece else 1.0
                if tail and pidx % 2 == 1:
                    nc.scalar.activation(
                        out=ysb[:], in_=yps[:],
                        func=mybir.ActivationFunctionType.Identity,
                        scale=scale,
                    )
                elif fp8piece:
                    nc.vector.tensor_scalar(
                        out=ysb[:], in0=yps[:], scalar1=scale,
                        scalar2=None, op0=mybir.AluOpType.mult,
                    )
                else:
                    nc.vector.tensor_copy(out=ysb[:], in_=yps[:])
                row0 = qb * QB + qs * 128
                dma_eng = nc.gpsimd if (tail and pidx % 2 == 1) else nc.sync
                dma_eng.dma_start(
                    out=y[row0:row0 + 128, dc * QB:(dc + 1) * QB], in_=ysb[:]
                )

            def emit_back(qb, h, jp, npairs):
                fp8blk = causal and qb >= QB8_START
                if jp == 0:
                    o_psum[qb, h] = o_ps.tile([128, QB], F32, tag="o",
                                              name=f"o_{qb}_{h}")
                    r_psum[qb, h] = r_ps.tile([128, QB], F32, tag="r",
                                              name=f"r_{qb}_{h}")
                pt, qoff0 = pts.pop((qb, h, jp))
                if fp8blk:
                    # fp8 DoubleRow pair passes: PV and rowsum cover both key
                    # blocks in one 256-deep pass each.  The rowsum goes
                    # first: at head boundaries the o bank may still be held
                    # by the previous head's normalize, and the r-bank pass
                    # hides part of that wait.
                    pt2_ap = pt[:].rearrange("p (j q) -> p j q", j=2)
                    nc.tensor.matmul(
                        r_psum[qb, h][:, qoff0:],
                        lhsT=ones8[:].rearrange("p (j c) -> p j c", j=2),
                        rhs=pt2_ap[:, :, qoff0:],
                        start=(jp == 0),
                        stop=(jp == npairs - 1),
                        perf_mode=DR,
                    )
                    nc.tensor.matmul(
                        o_psum[qb, h][:, qoff0:],
                        lhsT=v8_sb[h][:, jp * 256:(jp + 1) * 256].rearrange(
                            "p (j d) -> p j d", j=2),
                        rhs=pt2_ap[:, :, qoff0:],
                        start=(jp == 0),
                        stop=(jp == npairs - 1),
                        perf_mode=DR,
                    )
                else:
                    for j in range(2):
                        nkb = 2 * jp + j
                        qoff = qoff_of(qb, nkb)
                        w = QB - qoff
                        nc.tensor.matmul(
                            r_psum[qb, h][:, qoff:],
                            lhsT=ones[:],
                            rhs=pt[j][:, :w],
                            start=(nkb == 0),
                            stop=(nkb == 2 * npairs - 1),
                        )
                        nc.tensor.matmul(
                            o_psum[qb, h][:, qoff:],
                            lhsT=v_sb[h][:, nkb * KB:(nkb + 1) * KB],
                            rhs=pt[j][:, :w],
                            start=(nkb == 0),
                            stop=(nkb == 2 * npairs - 1),
                        )
                if jp == npairs - 1:
                    # fast approx reciprocal straight from PSUM (frees the r
                    # bank in one op), then one DVE multiply normalizes oT
                    rbc = small.tile([128, QB], F32, tag="rbc",
                                     name=f"rb_{qb}_{h}")
                    nc.vector.reciprocal_approx_fast(
                        out=rbc[:], in_=r_psum.pop((qb, h))[:])
                    if causal and qb >= OPROJ8_START:
                        # both heads' normalized oT land in one fp8 tile so
                        # the output projection runs as a single DoubleRow
                        # pass (256-deep contraction: h0+h1)
                        if h == 0:
                            oT_sb[qb] = otpool.tile(
                                [128, 2 * QB], F8, tag="ot8", name=f"ot8_{qb}")
                        nc.vector.tensor_mul(
                            oT_sb[qb][:, h * QB:(h + 1) * QB],
                            o_psum.pop((qb, h))[:], rbc[:])
                    else:
                        ot = otpool.tile([128, QB], F16, tag="ot",
                                         name=f"ot_{qb}_{h}")
                        nc.vector.tensor_mul(
                            ot[:], o_psum.pop((qb, h))[:], rbc[:])
                        oT_sb[qb, h] = ot
                    if h == HPC - 1:
                        ready.append(qb)

            ready_at = {}
            pidx = 0
            for i, u in enumerate(units):
                pts[u[:3]] = emit_front(*u)
                if i >= SKEW:
                    n_ready = len(ready)
                    emit_back(*units[i - SKEW])
                    if len(ready) > n_ready:
                        ready_at[ready[-1]] = i
                while ready and i - ready_at[ready[0]] >= DELAY:
                    qb = ready.pop(0)
                    for qs in range(QB // 128):
                        for dc in range(D // QB):
                            pending.append((qb, qs, dc))
                # drip outproj pieces between pairs to spread the load
                for _ in range(2):
                    if pending:
                        emit_outproj_piece(*pending.popleft(), pidx)
                        pidx += 1
            for u in units[-SKEW:]:
                emit_back(*u)
                for _ in range(2):
                    if pending:
                        emit_outproj_piece(*pending.popleft(), pidx)
                        pidx += 1
            # keep the PE (and its HAM clock gate) busy while the final
            # head's rowsum-reciprocal chain resolves
            warm2 = y_ps.tile([128, QB], F32, tag="y", name="warm2")
            for wi in range(6):
                nc.tensor.matmul(
                    warm2[:], lhsT=ones[:], rhs=warm_sb[:],
                    start=(wi == 0), stop=(wi == 5),
                )
            for qb in ready:
                for qs in range(QB // 128):
                    for dc in range(D // QB):
                        pending.append((qb, qs, dc))
            while pending:
                emit_outproj_piece(*pending.popleft(), pidx, tail=True)
                pidx += 1

    nc.compile()
    return nc


_NC_CACHE: dict = {}


def _get_nc(causal: bool) -> bass.Bass:
    if causal not in _NC_CACHE:
        _NC_CACHE[causal] = build_nc(causal)
    return _NC_CACHE[causal]


def _e4m3(a):
    return np.clip(a, -240.0, 240.0).astype(ml_dtypes.float8_e4m3)


def _pack_w(wT):
    # [(c j p), d] -> [(c p), (j d)]: the SBUF weight-chunk tile layout
    return np.ascontiguousarray(
        wT.reshape(KC8, 2, 128, CD).transpose(0, 2, 1, 3).reshape(
            KC8 * 128, 2 * CD))


def _pack_wo(woT):
    # [(h p), d] -> [p, (h d)]
    return np.ascontiguousarray(
        woT.reshape(HPC, 128, D).transpose(1, 0, 2).reshape(128, HPC * D))


def _make_in_maps(x, attn_mask, Wq, bq, Wk, bk, Wv, bv, Wo, bo, causal):
    xT = np.ascontiguousarray(x.T).astype(np.float16)
    # xT8 packed per (chunk, stripe) tile: [(c nb p), (j q)]
    xT8 = np.ascontiguousarray(
        _e4m3(x.T).reshape(KC8, 2, 128, NQB, QB).transpose(0, 3, 2, 1, 4)
        .reshape(KC8 * NQB * 128, 2 * QB))
    # fp16 x packed per (stripe, chunk) tile: [(nb c p), (g q)]
    xT16 = np.ascontiguousarray(
        xT[:, :V8_STRIPE_START * QB]
        .reshape(KC8, 2, 128, V8_STRIPE_START, QB).transpose(3, 0, 2, 1, 4)
        .reshape(V8_STRIPE_START * KC8 * 128, 2 * QB))
    maskT = None if causal else np.ascontiguousarray(attn_mask.T)
    in_maps = []
    for c in range(NCORES):
        sl = slice(c * CD, (c + 1) * CD)
        m = {
            "xT": xT16,
            "xT8": xT8,
            "wqT8": _pack_w(_e4m3(np.ascontiguousarray(Wq[sl, :].T) * W8SCALE)),
            "wkT8": _pack_w(_e4m3(np.ascontiguousarray(Wk[sl, :].T) * W8SCALE)),
            "wvT": _pack_w(np.ascontiguousarray(Wv[sl, :].T).astype(np.float16)),
            "wvT8": _pack_w(_e4m3(np.ascontiguousarray(Wv[sl, :].T) * W8SCALE)),
            "woT": _pack_wo(np.ascontiguousarray(Wo[:, sl].T).astype(np.float16)),
            "woT8": _pack_wo(_e4m3(np.ascontiguousarray(Wo[:, sl].T) * W8SCALE)),
            "bq": np.ascontiguousarray(bq[sl]),
            "bk": np.ascontiguousarray(bk[sl]),
        }
        if maskT is not None:
            m["maskT"] = maskT
        in_maps.append(m)
    return in_maps


def _is_causal(attn_mask) -> bool:
    if attn_mask.shape != (N, N):
        return False
    expected = np.where(
        np.tril(np.ones((N, N), dtype=bool)), np.float32(0.0), np.float32(NEG)
    )
    return bool(np.array_equal(attn_mask, expected))


def run_spmd(in_maps, causal, **kwargs):
    nc = _get_nc(causal)
    return run_bass_kernel_spmd(nc, in_maps, core_ids=list(range(NCORES)), **kwargs)


def kernel(x, attn_mask, Wq, bq, Wk, bk, Wv, bv, Wo, bo):
    causal = _is_causal(np.asarray(attn_mask))
    in_maps = _make_in_maps(
        np.asarray(x, np.float32), np.asarray(attn_mask, np.float32),
        np.asarray(Wq, np.float32), np.asarray(bq, np.float32),
        np.asarray(Wk, np.float32), np.asarray(bk, np.float32),
        np.asarray(Wv, np.float32), np.asarray(bv, np.float32),
        np.asarray(Wo, np.float32), np.asarray(bo, np.float32),
        causal,
    )
    res = run_spmd(in_maps, causal)
    # v's bias contribution folds exactly through the output projection
    # (attention rows sum to 1):  y += bo + Wo @ bv
    out = np.broadcast_to(
        (np.asarray(bo, np.float32)
         + np.asarray(Wo, np.float32) @ np.asarray(bv, np.float32)), (N, D)
    ).copy()
    for r in res.results:
        out += r["y"].astype(np.float32)
    return out


# revision 64
# speedup vs baseline: 1.2409x; 1.0119x over previous
"""Causal self-attention TRN2 kernel, tensor-parallel over heads on 8 NeuronCores.

Model (N=4096 tokens, D=2048, H=16 heads, HD=128):
    q = x @ Wq.T + bq ; k = x @ Wk.T + bk ; v = x @ Wv.T + bv   (per head)
    attn = softmax(q k^T / sqrt(HD) + causal_mask)
    y = concat_h(attn @ v) @ Wo.T + bo
Sharding: core c owns heads {2c, 2c+1} -> computes its QKV column slices,
attention for its heads, and a partial output projection
y_c = out_heads_c @ Wo[:, cols_c].T (+ bias/8).  Host sums the 8 partials.

Per-core kernel layout choices:
  * x is fed transposed (xT: D x N, fp16); an fp8 copy is cast on-device
    (DVE) for the Q/K projections, which run entirely in fp8e4 DoubleRow
    (256-deep contraction, 2x PE rate).  V stays fp16: fp8 V-projection
    noise passes straight through peaked attention rows (measured 3e-2).
  * q,k are produced directly transposed per head: qT/kT = (HD x N), fp16.
  * scores are computed transposed: sT[k,q] = kT_blk.T @ qT_blk, so the
    PV matmul needs no transposes at all: oT += v_blk.T @ exp(sT).
  * softmax skips the max-subtraction (scores are O(1); exp cannot
    overflow) -> row sums come from a ones-vector matmul on the PE, and
    1/rowsum is applied to oT (broadcast along partitions).
  * causality: key blocks entirely above the diagonal are skipped; blocks
    straddling the diagonal get -1e9 added via a precomputed triangular
    strip before the exp.
  * the attention pipeline works in key-block PAIRS: the two score matmuls
    of a pair share one 2-bank PSUM tile, so exp runs as a single 1024-wide
    ACT instruction (the ACT engine is the phase-2 co-bottleneck).
  * for query blocks >= QB8_START both exp(scores) and v are kept in fp8,
    so PV and the row-sum run as DoubleRow pair passes (2 key blocks per
    512-cycle pass).  Early rows (peaked attention) stay fp16.
  * v bias folds into the output bias exactly (attn rows sum to 1):
    y += (bo + Wo @ bv) / ncores  added on-device per core.
  * y partials leave the device as fp16 (host accumulates in fp32).
"""

from collections import deque
from contextlib import ExitStack

import numpy as np
import ml_dtypes

import concourse.bass as bass
import concourse.tile as tile
from concourse import bacc
from concourse import mybir
from concourse.bass_utils import run_bass_kernel_spmd
from concourse.masks import make_identity

N, D, H, HD = 4096, 2048, 16, 128
NCORES = 8
HPC = H // NCORES            # heads per core (2)
CD = HPC * HD                # per-core head-dim slice (256)
SCALE = 1.0 / float(np.sqrt(HD))
NEG = -1e9
W8SCALE = 16.0               # power-of-2 prescale keeping fp8 weights normal

QB = 512                     # query block (free dim of moving operands)
KB = 128                     # key block (partition dim of scores)
NQB = N // QB                # 8
KC8 = D // 256               # 256-deep contraction chunks (8)

F32 = mybir.dt.float32
F16 = mybir.dt.float16
F8 = mybir.dt.float8e4
DR = mybir.MatmulPerfMode.DoubleRow

# Query blocks >= QB8_START keep exp(scores) and v in fp8e4 so PV and the
# row-sum matmuls run in DoubleRow mode (2 key blocks per pass).  Early rows
# have peaked attention where quantization hurts; late rows average over many
# keys, so e4m3 noise washes out.
QB8_START = 1
# Query blocks >= OPROJ8_START run the output projection as a single fp8e4
# DoubleRow pass (256-deep: both heads at once).  The max error lives in the
# early rows (QK fp8 noise through peaked attention); fp8 outproj noise on
# later rows stays below it (verified against the reference inputs).
OPROJ8_START = 1
# V projection runs fp8e4 DoubleRow for token stripes >= 1: those v rows are
# only ever consumed through the (already fp8) v8 path by diffuse late query
# rows.  Stripe 0 keeps the fp16 projection for the peaked early rows.
V8_STRIPE_START = 1


def build_nc(causal: bool = True) -> bass.Bass:
    nc = bacc.Bacc(None)

    # All bulk inputs are host-packed into the exact SBUF tile layouts, so
    # every DMA below is a flat contiguous row-slice (one max-size packet
    # per partition -- no strided descriptors).
    # fp16 x is only needed for the fp16 V projection of the first
    # V8_STRIPE_START token stripes; everything else consumes the fp8 copy.
    xT = nc.declare_dram_parameter(
        "xT", [V8_STRIPE_START * KC8 * 128, 2 * QB], F16, isOutput=False)
    xT8 = nc.declare_dram_parameter(
        "xT8", [KC8 * NQB * 128, 2 * QB], F8, isOutput=False)
    wqT8 = nc.declare_dram_parameter("wqT8", [KC8 * 128, 2 * CD], F8, isOutput=False)
    wkT8 = nc.declare_dram_parameter("wkT8", [KC8 * 128, 2 * CD], F8, isOutput=False)
    wvT = nc.declare_dram_parameter("wvT", [KC8 * 128, 2 * CD], F16, isOutput=False)
    wvT8 = nc.declare_dram_parameter("wvT8", [KC8 * 128, 2 * CD], F8, isOutput=False)
    woT = nc.declare_dram_parameter("woT", [128, HPC * D], F16, isOutput=False)
    woT8 = nc.declare_dram_parameter("woT8", [128, HPC * D], F8, isOutput=False)
    bq = nc.declare_dram_parameter("bq", [CD], F32, isOutput=False)
    bk = nc.declare_dram_parameter("bk", [CD], F32, isOutput=False)
    maskT = None
    if not causal:
        maskT = nc.declare_dram_parameter("maskT", [N, N], F32, isOutput=False)
    y = nc.declare_dram_parameter("y", [N, D], F16, isOutput=True)

    # fp16 v is only consumed by the fp16 PV path (query blocks < QB8_START),
    # which under causality only ever reads keys < QB8_START * QB.
    NV16 = QB8_START * QB if causal else N

    with tile.TileContext(nc) as tc, tc.tile_pool(name="persist", bufs=1) as persist:
        # ---------------- setup: weights, biases, constants -------------
        # Wo^T slice: (CD, D) -> per head (128, D); DMA'd later (scalar
        # queue, after the projection weights -- it is first needed at the
        # output projection, deep into phase 2).  fp16 for early query
        # blocks, fp8 (DoubleRow, both heads per pass) for the rest.
        wo_sb = persist.tile([128, HPC * D], F16, tag="wo")
        wo8_sb = persist.tile([128, HPC * D], F8, tag="wo8")
        # q/k biases: (CD,) -> (128, HPC), partition = dim within head
        bq_sb = persist.tile([128, HPC], F32, tag="bq")
        bk_sb = persist.tile([128, HPC], F32, tag="bk")
        nc.sync.dma_start(out=bq_sb[:], in_=bq[:].rearrange("(h p) -> p h", p=128))
        nc.sync.dma_start(out=bk_sb[:], in_=bk[:].rearrange("(h p) -> p h", p=128))
        # identity for PE transposes
        ident = persist.tile([128, 128], F16, tag="ident")
        # full ones matrix: row-sum matmul with this stationary operand
        # broadcasts the sum to all 128 output partitions at no extra cost
        ones = persist.tile([128, 128], F16, tag="ones")
        nc.vector.memset(ones[:], 1.0)
        # fp8 ones pair for DoubleRow row-sum matmuls (256-deep contraction)
        ones8 = persist.tile([128, 256], F8, tag="ones8")
        nc.vector.memset(ones8[:], 1.0)
        def emit_setup_selects():
            make_identity(nc, ident[:])

        # PE warm-up: dependency-free matmuls fill the DMA-startup window
        # and push the HAM clock gate to full rate before real work
        warm_sb = persist.tile([128, QB], F16, tag="warm")
        nc.vector.memset(warm_sb[:], 0.0)

        # Persistent activations: qT/kT per head (HD x N) fp16; v per head
        # stored (128, nblk*128 + hd) i.e. key-block-major with hd minor, in
        # fp16 (first NV16 keys only) and fp8 (all keys, feeds DR PV pairs).
        qT = [persist.tile([128, N], F16, tag=f"qT{h}", name=f"qT{h}")
              for h in range(HPC)]
        kT = [persist.tile([128, N], F16, tag=f"kT{h}", name=f"kT{h}")
              for h in range(HPC)]
        v_sb = [persist.tile([128, NV16], F16, tag=f"v{h}", name=f"v{h}")
                for h in range(HPC)]
        v8_sb = [persist.tile([128, N], F8, tag=f"v8{h}", name=f"v8{h}")
                 for h in range(HPC)]

        # ---------------- phase 1: QKV projections ----------------------
        # Q/K run in fp8e4 DoubleRow mode (256-deep contraction per pass,
        # 2x PE rate) on a device-cast fp8 copy of x; V stays fp16.
        with ExitStack() as p1:
            wproj = p1.enter_context(tc.tile_pool(name="wproj", bufs=1))
            xin = p1.enter_context(tc.tile_pool(name="xin", bufs=8))
            x8in = p1.enter_context(tc.tile_pool(name="x8in", bufs=8))
            vtpool = p1.enter_context(tc.tile_pool(name="vt", bufs=2))
            qkv_ps = p1.enter_context(tc.tile_pool(name="qkv_ps", bufs=6, space="PSUM"))
            tp_ps = p1.enter_context(tc.tile_pool(name="tp_ps", bufs=2, space="PSUM"))

            # Per-chunk weight tiles keep reader deps fine-grained: the first
            # matmul of chunk c only waits on chunk c's DMA, not the whole
            # weight array.  fp8 Q/K chunks [p, (j d)], contraction row
            # j*128+p; fp16 V chunks: two 128-deep subchunks [p, (g d)].
            wq8_c = [wproj.tile([128, 2 * CD], F8, tag=f"wq8_{c}", name=f"wq8_{c}")
                     for c in range(KC8)]
            wk8_c = [wproj.tile([128, 2 * CD], F8, tag=f"wk8_{c}", name=f"wk8_{c}")
                     for c in range(KC8)]
            wv8_c = [wproj.tile([128, 2 * CD], F8, tag=f"wv8_{c}", name=f"wv8_{c}")
                     for c in range(KC8)]
            wv_c = [wproj.tile([128, 2 * CD], F16, tag=f"wv_{c}", name=f"wv_{c}")
                    for c in range(KC8)]
            # weight DMAs go on the scalar queue so the sync queue is free to
            # carry half of the x stream from the very start
            for c in range(KC8):
                for w_sb, w_dram in ((wq8_c[c], wqT8), (wk8_c[c], wkT8),
                                     (wv8_c[c], wvT8), (wv_c[c], wvT)):
                    nc.scalar.dma_start(
                        out=w_sb[:],
                        in_=w_dram[c * 128:(c + 1) * 128, :],
                    )
            warm_ps = tp_ps.tile([128, QB], F32, tag="tp", name="warm_ps")
            for wi in range(20):
                nc.tensor.matmul(
                    warm_ps[:], lhsT=ones[:], rhs=warm_sb[:],
                    start=(wi == 0), stop=(wi == 19),
                )

            for nb in range(N // QB):  # 512-token stripes
                # psum tiles for qT/kT/vT of both heads
                pss = {}
                for nm in ("q", "k", "v"):
                    for h in range(HPC):
                        pss[nm, h] = qkv_ps.tile(
                            [128, QB], F32, tag="qkv", name=f"ps_{nm}{h}_{nb}"
                        )
                wnames = (((wq8_c, "q"), (wk8_c, "k"), (wv8_c, "v"))
                          if nb >= V8_STRIPE_START
                          else ((wq8_c, "q"), (wk8_c, "k")))
                for c in range(KC8):  # 256-deep contraction chunks
                    # alternate x chunks across two DMA queues for bandwidth
                    dma_eng = nc.gpsimd if c % 2 == 0 else nc.sync
                    x8t = x8in.tile([128, 2 * QB], F8, tag="x8",
                                    name=f"x8_{nb}_{c}")
                    r0 = (c * NQB + nb) * 128
                    dma_eng.dma_start(out=x8t[:], in_=xT8[r0:r0 + 128, :])
                    x8_ap = x8t[:].rearrange("p (j q) -> p j q", j=2)
                    for w_c, nm in wnames:
                        w_ap = w_c[c][:].rearrange("p (j d) -> p j d", j=2)
                        for h in range(HPC):
                            nc.tensor.matmul(
                                pss[nm, h][:],
                                lhsT=w_ap[:, :, h * HD:(h + 1) * HD],
                                rhs=x8_ap,
                                start=(c == 0),
                                stop=(c == KC8 - 1),
                                perf_mode=DR,
                            )
                if nb < V8_STRIPE_START:
                    # fp16 V runs as a second sweep so the fp8 Q/K stream
                    # (whose x8 tiles land first) is never blocked on the
                    # bigger fp16 x transfers during the DMA ramp
                    for c in range(KC8):
                        xt = xin.tile([128, 2 * QB], F16, tag="xt",
                                      name=f"xt_{nb}_{c}")
                        dma_eng = nc.gpsimd if c % 2 == 0 else nc.sync
                        r0 = (nb * KC8 + c) * 128
                        dma_eng.dma_start(out=xt[:], in_=xT[r0:r0 + 128, :])
                        for g in range(2):
                            for h in range(HPC):
                                nc.tensor.matmul(
                                    pss["v", h][:],
                                    lhsT=wv_c[c][:, g * CD + h * HD:
                                                 g * CD + (h + 1) * HD],
                                    rhs=xt[:, g * QB:(g + 1) * QB],
                                    start=(c == 0 and g == 0),
                                    stop=(c == KC8 - 1 and g == 1),
                                )
                if nb == 0:
                    # gpsimd setup ops, emitted after the first stripe's x DMAs
                    # so they don't block the queue head (ident is first needed
                    # by the v transposes just below)
                    emit_setup_selects()
                for nm, b_sb, dst in (("q", bq_sb, qT), ("k", bk_sb, kT)):
                    for h in range(HPC):
                        # 1/W8SCALE undoes the fp8 weight prescale
                        nc.scalar.activation(
                            out=dst[h][:, nb * QB:(nb + 1) * QB],
                            in_=pss[nm, h][:],
                            func=mybir.ActivationFunctionType.Identity,
                            bias=b_sb[:, h:h + 1],
                            scale=1.0 / W8SCALE,
                        )
                # v: evacuate vT (undoing the fp8 prescale for DR stripes),
                # then PE-transpose into (n, hd) layout, stored fp8 (all
                # keys) + fp16 (early keys)
                for h in range(HPC):
                    vt = vtpool.tile([128, QB], F16, tag="vt", name=f"vt_{nb}_{h}")
                    if nb >= V8_STRIPE_START:
                        nc.vector.tensor_scalar(
                            out=vt[:], in0=pss["v", h][:],
                            scalar1=1.0 / W8SCALE, scalar2=None,
                            op0=mybir.AluOpType.mult,
                        )
                    else:
                        nc.vector.tensor_copy(out=vt[:], in_=pss["v", h][:])
                    for s in range(QB // 128):
                        tp = tp_ps.tile([128, 128], F16, tag="tp",
                                        name=f"tp_{nb}_{h}_{s}")
                        nc.tensor.transpose(
                            tp[:], vt[:, s * 128:(s + 1) * 128], ident[:]
                        )
                        nblk = nb * (QB // 128) + s
                        nc.vector.tensor_copy(
                            out=v8_sb[h][:, nblk * 128:(nblk + 1) * 128],
                            in_=tp[:],
                        )
                        if nblk * 128 < NV16:
                            nc.vector.tensor_copy(
                                out=v_sb[h][:, nblk * 128:(nblk + 1) * 128],
                                in_=tp[:],
                            )

        # wo weight DMAs submit at the start of phase 2: they are first
        # needed ~30us later and would otherwise steal HBM bandwidth from
        # the x stream during the phase-1 ramp
        nc.scalar.dma_start(out=wo_sb[:], in_=woT[:])
        nc.scalar.dma_start(out=wo8_sb[:], in_=woT8[:])

        # ---------------- phase 2: attention + output projection --------
        with ExitStack() as p2:
            ptpool = p2.enter_context(tc.tile_pool(name="pt", bufs=8))
            otpool = p2.enter_context(tc.tile_pool(name="ot", bufs=6))
            ypool = p2.enter_context(tc.tile_pool(name="yout", bufs=12))
            small = p2.enter_context(tc.tile_pool(name="small", bufs=4))
            mtpool = p2.enter_context(tc.tile_pool(name="mt", bufs=4))
            # score-pair tiles: [128, 1024] fp32 = 2 PSUM banks each, so one
            # 1024-wide ACT exp covers both key blocks of a pair
            s_ps = p2.enter_context(tc.tile_pool(name="s_ps", bufs=2, space="PSUM"))
            o_ps = p2.enter_context(tc.tile_pool(name="o_ps", bufs=1, space="PSUM"))
            r_ps = p2.enter_context(tc.tile_pool(name="r_ps", bufs=1, space="PSUM"))
            y_ps = p2.enter_context(tc.tile_pool(name="y_ps", bufs=2, space="PSUM"))

            # Flat software-pipelined attention over key-block PAIRS:
            # scores/exp run SKEW pairs ahead of PV/rowsum, so the PE never
            # waits on the ACT exp latency (or the GpSimd diagonal-select
            # latency) -- including across head and query-block boundaries.
            # Output-projection pieces drip in between pairs to spread load.
            SKEW = 4
            units = []   # (qb, h, jp, npairs)
            for qb in range(NQB):
                npairs = (qb + 1) * (QB // KB) // 2 if causal else N // KB // 2
                for h in range(HPC):
                    for jp in range(npairs):
                        units.append((qb, h, jp, npairs))
            ready = []      # qblocks whose heads are normalized
            DELAY = 4       # pipeline pairs between normalize and outproj
            pending = deque()

            pts = {}
            o_psum = {}
            r_psum = {}
            oT_sb = {}

            def qoff_of(qb, nkb):
                # causal: columns q < off are fully masked for this key block;
                # skip them (exact -- their exp is 0)
                off = nkb * KB - qb * QB
                return max(0, off) if causal else 0

            def emit_front(qb, h, jp, npairs):
                fp8blk = causal and qb >= QB8_START
                sps = s_ps.tile([128, 2 * QB], F32, tag="s",
                                name=f"s_{qb}_{h}_{jp}")
                qoffs = []
                for j in range(2):
                    nkb = 2 * jp + j
                    qoff = qoff_of(qb, nkb)
                    qoffs.append(qoff)
                    w = QB - qoff
                    half = sps[:, j * QB:(j + 1) * QB]
                    nc.tensor.matmul(
                        half[:, qoff:],
                        lhsT=kT[h][:, nkb * KB:(nkb + 1) * KB],
                        rhs=qT[h][:, qb * QB + qoff:(qb + 1) * QB],
                        start=True,
                        stop=True,
                    )
                    if not causal:
                        mt = mtpool.tile([128, QB], F32, tag="mt",
                                         name=f"mt_{qb}_{h}_{jp}_{j}")
                        nc.sync.dma_start(
                            out=mt[:],
                            in_=maskT[nkb * KB:(nkb + 1) * KB,
                                      qb * QB:(qb + 1) * QB],
                        )
                        nc.vector.scalar_tensor_tensor(
                            out=half[:],
                            in0=mt[:],
                            scalar=1.0 / SCALE,
                            in1=half[:],
                            op0=mybir.AluOpType.mult,
                            op1=mybir.AluOpType.add,
                        )
                qoff0, qoff1 = qoffs
                if fp8blk:
                    pt2 = ptpool.tile([128, 2 * QB], F8, tag="pt8",
                                      name=f"pt8_{qb}_{h}_{jp}")
                    # one fused exp covers the whole pair (columns < qoff0
                    # are fully masked for both blocks -- skipped); causal
                    # masking of diagonal blocks happens AFTER the exp as a
                    # triangular zero-fill on the fp8 tile (GpSimd, which is
                    # idle in phase 2 -- keeps ACT and DVE off this path)
                    nc.scalar.activation(
                        out=pt2[:, qoff0:], in_=sps[:, qoff0:],
                        func=mybir.ActivationFunctionType.Exp,
                        scale=SCALE,
                    )
                    for j in range(2):
                        off = (2 * jp + j) * KB - qb * QB
                        if causal and off >= 0:
                            nc.gpsimd.affine_select(
                                out=pt2[:, j * QB + qoff0:(j + 1) * QB],
                                in_=pt2[:, j * QB + qoff0:(j + 1) * QB],
                                compare_op=mybir.AluOpType.is_ge,
                                fill=0.0,
                                base=qoff0 - off,
                                pattern=[[1, QB - qoff0]],
                                channel_multiplier=-1,
                            )
                    return (pt2, qoff0)
                # fp16 path: two per-block exps into separate fp16 tiles
                res = []
                for j in range(2):
                    nkb = 2 * jp + j
                    qoff = qoffs[j]
                    w = QB - qoff
                    pt = ptpool.tile([128, QB], F16, tag="pt",
                                     name=f"pt_{qb}_{h}_{jp}_{j}")
                    nc.scalar.activation(
                        out=pt[:, :w], in_=sps[:, j * QB + qoff:(j + 1) * QB],
                        func=mybir.ActivationFunctionType.Exp,
                        scale=SCALE,
                    )
                    if causal and nkb * KB - qb * QB >= 0:
                        nc.gpsimd.affine_select(
                            out=pt[:, :w],
                            in_=pt[:, :w],
                            compare_op=mybir.AluOpType.is_ge,
                            fill=0.0,
                            base=0,
                            pattern=[[1, w]],
                            channel_multiplier=-1,
                        )
                    res.append(pt)
                return (res, None)

            def emit_outproj_piece(qb, qs, dc, pidx, tail=False):
                yps = y_ps.tile([128, QB], F32, tag="y",
                                name=f"y_{qb}_{qs}_{dc}")
                fp8piece = causal and qb >= OPROJ8_START
                if fp8piece:
                    # single DoubleRow pass: contraction over both heads'
                    # 128 oT dims at once (fp8 oT x fp8 Wo)
                    nc.tensor.matmul(
                        yps[:],
                        lhsT=oT_sb[qb][:].rearrange(
                            "p (j q) -> p j q", j=2)[:, :, qs * 128:(qs + 1) * 128],
                        rhs=wo8_sb[:].rearrange(
                            "p (j d) -> p j d", j=2)[:, :, dc * QB:(dc + 1) * QB],
                        start=True,
                        stop=True,
                        perf_mode=DR,
                    )
                else:
                    for h in range(HPC):
                        nc.tensor.matmul(
                            yps[:],
                            lhsT=oT_sb[qb, h][:, qs * 128:(qs + 1) * 128],
                            rhs=wo_sb[:, h * D + dc * QB: h * D + (dc + 1) * QB],
                            start=(h == 0),
                            stop=(h == HPC - 1),
                        )
                ysb = ypool.tile([128, QB], F16, tag="ysb",
                                 name=f"ys_{qb}_{qs}_{dc}")
                # psum evacuation on DVE (output bias is added on the host;
                # fp8 pieces also undo the Wo prescale); during the final
                # flush the exp stream is done, so ACT shares the evacuation
                # load and the y DMAs fan out over two queues
                scale = 1.0 / W8SCALE if fp8piece else 1.0
                if tail and pidx % 2 == 1:
                    nc.scalar.activation(
                        out=ysb[:], in_=yps[:],
                        func=mybir.ActivationFunctionType.Identity,
                        scale=scale,
                    )
                elif fp8piece:
                    nc.vector.tensor_scalar(
                        out=ysb[:], in0=yps[:], scalar1=scale,
                        scalar2=None, op0=mybir.AluOpType.mult,
                    )
                else:
                    nc.vector.tensor_copy(out=ysb[:], in_=yps[:])
                row0 = qb * QB + qs * 128
                dma_eng = nc.gpsimd if (tail and pidx % 2 == 1) else nc.sync
                dma_eng.dma_start(
                    out=y[row0:row0 + 128, dc * QB:(dc + 1) * QB], in_=ysb[:]
                )

            def emit_back(qb, h, jp, npairs):
                fp8blk = causal and qb >= QB8_START
                if jp == 0:
                    o_psum[qb, h] = o_ps.tile([128, QB], F32, tag="o",
                                              name=f"o_{qb}_{h}")
                    r_psum[qb, h] = r_ps.tile([128, QB], F32, tag="r",
                                              name=f"r_{qb}_{h}")
                pt, qoff0 = pts.pop((qb, h, jp))
                if fp8blk:
                    # fp8 DoubleRow pair passes: PV and rowsum cover both key
                    # blocks in one 256-deep pass each.  The rowsum goes
                    # first: at head boundaries the o bank may still be held
                    # by the previous head's normalize, and the r-bank pass
                    # hides part of that wait.
                    pt2_ap = pt[:].rearrange("p (j q) -> p j q", j=2)
                    nc.tensor.matmul(
                        r_psum[qb, h][:, qoff0:],
                        lhsT=ones8[:].rearrange("p (j c) -> p j c", j=2),
                        rhs=pt2_ap[:, :, qoff0:],
                        start=(jp == 0),
                        stop=(jp == npairs - 1),
                        perf_mode=DR,
                    )
                    nc.tensor.matmul(
                        o_psum[qb, h][:, qoff0:],
                        lhsT=v8_sb[h][:, jp * 256:(jp + 1) * 256].rearrange(
                            "p (j d) -> p j d", j=2),
                        rhs=pt2_ap[:, :, qoff0:],
                        start=(jp == 0),
                        stop=(jp == npairs - 1),
                        perf_mode=DR,
                    )
                else:
                    for j in range(2):
                        nkb = 2 * jp + j
                        qoff = qoff_of(qb, nkb)
                        w = QB - qoff
                        nc.tensor.matmul(
                            r_psum[qb, h][:, qoff:],
                            lhsT=ones[:],
                            rhs=pt[j][:, :w],
                            start=(nkb == 0),
                            stop=(nkb == 2 * npairs - 1),
                        )
                        nc.tensor.matmul(
                            o_psum[qb, h][:, qoff:],
                            lhsT=v_sb[h][:, nkb * KB:(nkb + 1) * KB],
                            rhs=pt[j][:, :w],
                            start=(nkb == 0),
                            stop=(nkb == 2 * npairs - 1),
                        )
                if jp == npairs - 1:
                    # fast approx reciprocal straight from PSUM (frees the r
                    # bank in one op), then one DVE multiply normalizes oT
                    rbc = small.tile([128, QB], F32, tag="rbc",
                                     name=f"rb_{qb}_{h}")
                    nc.vector.reciprocal_approx_fast(
                        out=rbc[:], in_=r_psum.pop((qb, h))[:])
                    if causal and qb >= OPROJ8_START:
                        # both heads' normalized oT land in one fp8 tile so
                        # the output projection runs as a single DoubleRow
                        # pass (256-deep contraction: h0+h1)
                        if h == 0:
                            oT_sb[qb] = otpool.tile(
                                [128, 2 * QB], F8, tag="ot8", name=f"ot8_{qb}")
                        nc.vector.tensor_mul(
                            oT_sb[qb][:, h * QB:(h + 1) * QB],
                            o_psum.pop((qb, h))[:], rbc[:])
                    else:
                        ot = otpool.tile([128, QB], F16, tag="ot",
                                         name=f"ot_{qb}_{h}")
                        nc.vector.tensor_mul(
                            ot[:], o_psum.pop((qb, h))[:], rbc[:])
                        oT_sb[qb, h] = ot
                    if h == HPC - 1:
                        ready.append(qb)

            ready_at = {}
            pidx = 0
            for i, u in enumerate(units):
                pts[u[:3]] = emit_front(*u)
                if i >= SKEW:
                    n_ready = len(ready)
                    emit_back(*units[i - SKEW])
                    if len(ready) > n_ready:
                        ready_at[ready[-1]] = i
                while ready and i - ready_at[ready[0]] >= DELAY:
                    qb = ready.pop(0)
                    for qs in range(QB // 128):
                        for dc in range(D // QB):
                            pending.append((qb, qs, dc))
                # drip outproj pieces between pairs to spread the load; at
                # head boundaries drip extra pieces -- the independent PE
                # work covers the o/r-bank release latency (DVE reciprocal +
                # multiply) that the next head's first pair waits on
                prv = units[i - SKEW] if i >= SKEW else None
                ndrip = 4 if (prv and prv[2] == prv[3] - 1) else 2
                for _ in range(ndrip):
                    if pending:
                        emit_outproj_piece(*pending.popleft(), pidx)
                        pidx += 1
            for u in units[-SKEW:]:
                emit_back(*u)
                for _ in range(2):
                    if pending:
                        emit_outproj_piece(*pending.popleft(), pidx)
                        pidx += 1
            # keep the PE (and its HAM clock gate) busy while the final
            # head's rowsum-reciprocal chain resolves
            warm2 = y_ps.tile([128, QB], F32, tag="y", name="warm2")
            for wi in range(6):
                nc.tensor.matmul(
                    warm2[:], lhsT=ones[:], rhs=warm_sb[:],
                    start=(wi == 0), stop=(wi == 5),
                )
            for qb in ready:
                for qs in range(QB // 128):
                    for dc in range(D // QB):
                        pending.append((qb, qs, dc))
            while pending:
                emit_outproj_piece(*pending.popleft(), pidx, tail=True)
                pidx += 1

    nc.compile()
    return nc


_NC_CACHE: dict = {}


def _get_nc(causal: bool) -> bass.Bass:
    if causal not in _NC_CACHE:
        _NC_CACHE[causal] = build_nc(causal)
    return _NC_CACHE[causal]


def _e4m3(a):
    return np.clip(a, -240.0, 240.0).astype(ml_dtypes.float8_e4m3)


def _pack_w(wT):
    # [(c j p), d] -> [(c p), (j d)]: the SBUF weight-chunk tile layout
    return np.ascontiguousarray(
        wT.reshape(KC8, 2, 128, CD).transpose(0, 2, 1, 3).reshape(
            KC8 * 128, 2 * CD))


def _pack_wo(woT):
    # [(h p), d] -> [p, (h d)]
    return np.ascontiguousarray(
        woT.reshape(HPC, 128, D).transpose(1, 0, 2).reshape(128, HPC * D))


def _make_in_maps(x, attn_mask, Wq, bq, Wk, bk, Wv, bv, Wo, bo, causal):
    xT = np.ascontiguousarray(x.T).astype(np.float16)
    # xT8 packed per (chunk, stripe) tile: [(c nb p), (j q)]
    xT8 = np.ascontiguousarray(
        _e4m3(x.T).reshape(KC8, 2, 128, NQB, QB).transpose(0, 3, 2, 1, 4)
        .reshape(KC8 * NQB * 128, 2 * QB))
    # fp16 x packed per (stripe, chunk) tile: [(nb c p), (g q)]
    xT16 = np.ascontiguousarray(
        xT[:, :V8_STRIPE_START * QB]
        .reshape(KC8, 2, 128, V8_STRIPE_START, QB).transpose(3, 0, 2, 1, 4)
        .reshape(V8_STRIPE_START * KC8 * 128, 2 * QB))
    maskT = None if causal else np.ascontiguousarray(attn_mask.T)
    in_maps = []
    for c in range(NCORES):
        sl = slice(c * CD, (c + 1) * CD)
        m = {
            "xT": xT16,
            "xT8": xT8,
            "wqT8": _pack_w(_e4m3(np.ascontiguousarray(Wq[sl, :].T) * W8SCALE)),
            "wkT8": _pack_w(_e4m3(np.ascontiguousarray(Wk[sl, :].T) * W8SCALE)),
            "wvT": _pack_w(np.ascontiguousarray(Wv[sl, :].T).astype(np.float16)),
            "wvT8": _pack_w(_e4m3(np.ascontiguousarray(Wv[sl, :].T) * W8SCALE)),
            "woT": _pack_wo(np.ascontiguousarray(Wo[:, sl].T).astype(np.float16)),
            "woT8": _pack_wo(_e4m3(np.ascontiguousarray(Wo[:, sl].T) * W8SCALE)),
            "bq": np.ascontiguousarray(bq[sl]),
            "bk": np.ascontiguousarray(bk[sl]),
        }
        if maskT is not None:
            m["maskT"] = maskT
        in_maps.append(m)
    return in_maps


def _is_causal(attn_mask) -> bool:
    if attn_mask.shape != (N, N):
        return False
    expected = np.where(
        np.tril(np.ones((N, N), dtype=bool)), np.float32(0.0), np.float32(NEG)
    )
    return bool(np.array_equal(attn_mask, expected))


def run_spmd(in_maps, causal, **kwargs):
    nc = _get_nc(causal)
    return run_bass_kernel_spmd(nc, in_maps, core_ids=list(range(NCORES)), **kwargs)


def kernel(x, attn_mask, Wq, bq, Wk, bk, Wv, bv, Wo, bo):
    causal = _is_causal(np.asarray(attn_mask))
    in_maps = _make_in_maps(
        np.asarray(x, np.float32), np.asarray(attn_mask, np.float32),
        np.asarray(Wq, np.float32), np.asarray(bq, np.float32),
        np.asarray(Wk, np.float32), np.asarray(bk, np.float32),
        np.asarray(Wv, np.float32), np.asarray(bv, np.float32),
        np.asarray(Wo, np.float32), np.asarray(bo, np.float32),
        causal,
    )
    res = run_spmd(in_maps, causal)
    # v's bias contribution folds exactly through the output projection
    # (attention rows sum to 1):  y += bo + Wo @ bv
    out = np.broadcast_to(
        (np.asarray(bo, np.float32)
         + np.asarray(Wo, np.float32) @ np.asarray(bv, np.float32)), (N, D)
    ).copy()
    for r in res.results:
        out += r["y"].astype(np.float32)
    return out
